# revision 1
# baseline (speedup 1.0000x reference)
"""Multi-head causal attention block on 8 Trainium2 NeuronCores.

Sharding: tensor-parallel over heads (4 groups of 4 heads) x data-parallel
over batch (2). Core c -> (batch b=c//4, head-group g=c%4). Each core
computes q/k/v projections for its head group, causal attention for its 4
heads, and a partial output projection; the host sums the 4 partials per
batch. All layout transposes are done host-side so the device does none.

Self-contained: hardcodes shapes for the 2x2048x2048, 16-head problem.
"""

import os
from contextlib import ExitStack

import numpy as np

import concourse.bass as bass
import concourse.tile as tile
from concourse import bacc, mybir
from concourse.bass import ds, ts
from concourse.bass_utils import run_bass_kernel_spmd

F32 = mybir.dt.float32
F32R = mybir.dt.float32r
ACTF = mybir.ActivationFunctionType

# Full-problem dims
BATCH = 2
SEQ = 2048
D_MODEL = 2048
NUM_HEADS = 16
HEAD_DIM = 128
N_CORES = 8
N_GROUPS = 4  # head-groups (tensor parallel)
DG = D_MODEL // N_GROUPS  # 512 = 4 heads per group
SCALE = 1.0 / float(np.sqrt(HEAD_DIM))

QB = 512  # q-block width in attention
KT = 128  # k-tile width (partition dim)

USE_F32R = os.environ.get("KERNEL_F32", "0") != "1"
MMDT = F32R if USE_F32R else F32


def _r(ap):
    """View a float32 DRAM AP as the matmul dtype for DMA into MMDT tiles."""
    return ap.bitcast(F32R) if USE_F32R else ap


def _mha_body(ctx, tc, aps, S, D, DGl):
    """Per-core kernel body.

    aps: dict of DRAM APs: xt [D,S], wqt/wkt/wvt [D,DGl], wot [DGl,D],
      bq/bk [128, DGl//128], bv [128, DGl], bo [128, D], masks [4,128,QB],
      ones [128,1], out [S,D].

    k^T and v stay resident in SBUF (written directly by the projection
    drains); only q^T round-trips through DRAM.
    """
    nc = tc.nc
    n_kd = D // 128  # contraction tiles over d_model
    n_sq = S // QB  # 512-wide attention q-blocks
    n_sk = S // KT  # 128-wide seq tiles
    n_dg = DGl // 128  # head tiles per group
    QB1 = 256  # phase-1 seq-slice width
    n_ns = S // QB1

    xt, wqt, wkt, wvt, wot = aps["xt"], aps["wqt"], aps["wkt"], aps["wvt"], aps["wot"]
    out = aps["out"]

    # DRAM scratch for v [S, DGl] (q^T and k^T stay resident in SBUF)
    dram = ctx.enter_context(tc.tile_pool(name="dram", bufs=1, space="DRAM"))
    v_d = dram.tile([S, DGl], F32, name="v_d")

    consts = ctx.enter_context(tc.tile_pool(name="consts", bufs=1))
    # dummy activation first: forces the ACT function-table DMA to queue
    # ahead of the bulk input loads (else every early PSUM drain stalls)
    warm = consts.tile([128, 1], F32, name="act_warm")
    nc.vector.memset(warm[:], 0.0)
    nc.scalar.activation(warm[:], warm[:], ACTF.Identity, bias=warm[:, 0:1])
    ones_sb = consts.tile([128, 1], MMDT, name="ones_sb")
    bq_sb = consts.tile([128, n_dg], F32, name="bq_sb")
    bk_sb = consts.tile([128, n_dg], F32, name="bk_sb")
    bv_sb = consts.tile([128, DGl], F32, name="bv_sb")
    masks_sb = consts.tile([128, 4 * QB], F32, name="masks_sb")

    # resident q^T / k^T per head: [p, s] = q^T/k^T[h*128+p, s]
    kv_pool = ctx.enter_context(tc.tile_pool(name="kv_res", bufs=1))
    kt_res = [
        kv_pool.tile([128, S], MMDT, tag=f"ktr{h}", name=f"kt_res{h}")
        for h in range(n_dg)
    ]
    qt_res = [
        kv_pool.tile([128, S], MMDT, tag=f"qtr{h}", name=f"qt_res{h}")
        for h in range(n_dg)
    ]

    # ---------------- Phase 1: q/k/v projections ----------------
    with (
        tc.tile_pool(name="wqkv", bufs=1) as wpool,
        tc.tile_pool(name="xt_pool", bufs=2) as xpool,
        tc.tile_pool(name="p1_stage", bufs=2) as stage,
        tc.tile_pool(name="p1_psum", bufs=4, space="PSUM") as psum1,
    ):
        # weights resident: w*_sb[p, k*DGl + f] = w*t[k*128+p, f]
        w_sbs = {
            wname: wpool.tile([128, n_kd * DGl], MMDT, name=f"{wname}_sb")
            for wname in ("wq", "wk", "wv")
        }

        def load_w(wname, wap):
            nc.sync.dma_start(
                w_sbs[wname][:].rearrange("p (k f) -> p k f", k=n_kd),
                _r(wap).rearrange("(k p) f -> p k f", p=128),
            )

        def load_w_mblock(wname, wap, m):
            nc.sync.dma_start(
                w_sbs[wname][:].rearrange(
                    "p (k g j) -> p k g j", k=n_kd, j=128
                )[:, :, m, :],
                _r(wap).rearrange("(k p) (g j) -> p k g j", p=128, j=128)[
                    :, :, m, :
                ],
            )

        def load_xt(ns):
            # two k-half DMAs: the slice's first k-accumulations can start
            # as soon as the first half lands
            t = xpool.tile([128, n_kd * QB1], MMDT, tag="xt", name="xt_sb")
            half = n_kd // 2
            for hlf in range(2):
                nc.sync.dma_start(
                    t[:, ds(hlf * half * QB1, half * QB1)].rearrange(
                        "p (k f) -> p k f", k=half
                    ),
                    _r(
                        xt[ds(hlf * half * 128, half * 128), ts(ns, QB1)]
                    ).rearrange("(k p) f -> p k f", p=128),
                )
            return t

        nc.sync.dma_start(ones_sb[:], _r(aps["ones"]))
        nc.sync.dma_start(bq_sb[:], aps["bq"])
        nc.sync.dma_start(bk_sb[:], aps["bk"])
        nc.sync.dma_start(bv_sb[:], aps["bv"])
        # k^T first: PE can start on wk+x0 while wq/wv still stream in
        load_w_mblock("wk", wkt, 0)
        g0 = load_xt(0)
        for m in range(1, n_dg):
            load_w_mblock("wk", wkt, m)
        g1 = load_xt(1)
        for m in range(n_dg):
            load_w_mblock("wq", wqt, m)
        load_w("wv", wvt)
        nc.sync.dma_start(
            masks_sb[:].rearrange("p (i f) -> p i f", i=4),
            aps["masks"].rearrange("i p f -> p i f"),
        )

        def do_proj_t(res, wname, b_sb, ns, xt_sb):
            # q^T/k^T [m hd-dims 128, QB1 seq] drains into resident tiles
            for m in range(n_dg):
                ps = psum1.tile([128, QB1], F32, tag="ps", name="ps_qk")
                for k in range(n_kd):
                    nc.tensor.matmul(
                        ps[:],
                        lhsT=w_sbs[wname][:, ds(k * DGl + m * 128, 128)],
                        rhs=xt_sb[:, ts(k, QB1)],
                        start=(k == 0),
                        stop=(k == n_kd - 1),
                    )
                nc.scalar.activation(
                    res[m][:, ts(ns, QB1)],
                    ps[:],
                    ACTF.Identity,
                    bias=b_sb[:, ds(m, 1)],
                )

        def do_v(ns, xt_sb):
            for msub in range(QB1 // 128):
                ps = psum1.tile([128, DGl], F32, tag="ps", name="ps_v")
                for k in range(n_kd):
                    nc.tensor.matmul(
                        ps[:],
                        lhsT=xt_sb[:, ds(k * QB1 + msub * 128, 128)],
                        rhs=w_sbs["wv"][:, ts(k, DGl)],
                        start=(k == 0),
                        stop=(k == n_kd - 1),
                    )
                st = stage.tile([128, DGl], F32, tag="v_st", name="v_st")
                nc.vector.tensor_add(st[:], ps[:], bv_sb[:])
                nc.sync.dma_start(
                    v_d[ds(ns * QB1 + msub * 128, 128), :], st[:]
                )

        # head group: k^T for slices 0-1 (no DMA drains), then q^T, then v
        for ns, g in ((0, g0), (1, g1)):
            do_proj_t(kt_res, "wk", bk_sb, ns, g)
        for ns, g in ((0, g0), (1, g1)):
            do_proj_t(qt_res, "wq", bq_sb, ns, g)
        for ns, g in ((0, g0), (1, g1)):
            do_v(ns, g)
        nxt = load_xt(2) if n_ns > 2 else None
        for ns in range(2, n_ns):
            xt_sb = nxt
            nxt = load_xt(ns + 1) if ns + 1 < n_ns else None
            do_proj_t(kt_res, "wk", bk_sb, ns, xt_sb)
            do_proj_t(qt_res, "wq", bq_sb, ns, xt_sb)
            do_v(ns, xt_sb)

    # ---------------- Phase 2: causal attention ----------------
    # ctx^T per head stays resident in SBUF for phase 3
    ctx_pool = ctx.enter_context(tc.tile_pool(name="ctx_pool", bufs=1))
    ctx_sbs = [
        ctx_pool.tile([128, S], MMDT, tag=f"ctx{h}", name=f"ctx_sb{h}")
        for h in range(n_dg)
    ]

    # wo stays resident; loaded mid-phase-2 so phase 3 starts hot
    wopool = ctx.enter_context(tc.tile_pool(name="wo_pool", bufs=1))
    wo_sb = wopool.tile([128, n_dg * D], MMDT, name="wo_sb")

    # phase-2/3-only constants live after phase-1 pools are freed
    p2consts = ctx.enter_context(tc.tile_pool(name="p2consts", bufs=1))
    bo_sb = p2consts.tile([128, D], F32, name="bo_sb")
    nc.sync.dma_start(bo_sb[:], aps["bo"])

    with (
        tc.tile_pool(name="v_pool", bufs=3) as vpool,
        tc.tile_pool(name="exp_pool", bufs=8) as epool,
        tc.tile_pool(name="lrec_pool", bufs=3) as lpool,
        tc.tile_pool(name="bc_pool", bufs=3) as bcpool,
        tc.tile_pool(name="ps_s", bufs=5, space="PSUM") as ps_s_pool,
        tc.tile_pool(name="ps_c", bufs=2, space="PSUM") as ps_c_pool,
        tc.tile_pool(name="ps_l", bufs=1, space="PSUM") as ps_l_pool,
    ):
        for h in range(n_dg):
            # v_sb[p, t*128+j] = v[t*128+p, h*128+j]; quarter DMAs so the
            # first q-blocks' PV can start before the whole head lands
            v_sb = vpool.tile([128, n_sk * 128], MMDT, tag="v", name="v_sb")
            nq = max(1, S // 512)
            for vq in range(nq):
                nc.sync.dma_start(
                    v_sb[:, ds(vq * 512, 512)].rearrange(
                        "p (t j) -> p t j", j=128
                    ),
                    _r(v_d[ds(vq * 512, 512), ts(h, 128)]).rearrange(
                        "(t p) j -> p t j", p=128
                    ),
                )
            if h == 1:
                # wo_sb[p, k*D + f] = wot[k*128+p, f] (phase-3 prefetch)
                nc.sync.dma_start(
                    wo_sb[:].rearrange("p (k f) -> p k f", k=n_dg),
                    _r(wot).rearrange("(k p) f -> p k f", p=128),
                )
            for qb in range(n_sq):
                n_kt = (qb + 1) * (QB // KT)  # causal: only k-tiles <= q
                ps_c = ps_c_pool.tile([128, QB], F32, tag="c", name="ps_c")
                ps_l = ps_l_pool.tile([1, QB], F32, tag="l", name="ps_l")
                diag0 = n_kt - (QB // KT)
                for kt in range(n_kt):
                    off = kt - diag0
                    # causal column restriction: diagonal tile off needs
                    # only cols >= off*128; keep moving dim >= 256 for
                    # full-rate f32r (so off=3 starts at 256, masked).
                    sc = 0 if off < 1 else (128 if off == 1 else 256)
                    w = QB - sc
                    ps_sc = ps_s_pool.tile([128, QB], F32, tag="s", name="ps_sc")
                    nc.tensor.matmul(
                        ps_sc[:, ds(sc, w)],
                        lhsT=kt_res[h][:, ts(kt, 128)],
                        rhs=qt_res[h][:, ds(qb * QB + sc, w)],
                        start=True,
                        stop=True,
                    )
                    if off >= 0:
                        # only the triangular block (plus, for off=3, the
                        # fully-invalid 128 cols kept for moving-dim>=256)
                        # needs masking; columns right of it are all-valid
                        msc = off * 128 if off < 3 else 256
                        mw = 128 if off < 3 else 256
                        nc.vector.tensor_add(
                            ps_sc[:, ds(msc, mw)],
                            ps_sc[:, ds(msc, mw)],
                            masks_sb[:, ds(off * QB + msc, mw)],
                        )
                    ex = epool.tile([128, QB], MMDT, tag="e", name="ex")
                    nc.scalar.activation(
                        ex[:, ds(sc, w)], ps_sc[:, ds(sc, w)], ACTF.Exp, scale=SCALE
                    )
                    nc.tensor.matmul(
                        ps_c[:, ds(sc, w)],
                        lhsT=v_sb[:, ts(kt, 128)],
                        rhs=ex[:, ds(sc, w)],
                        start=(kt == 0),
                        stop=(kt == n_kt - 1),
                        skip_group_check=True,
                    )
                    nc.tensor.matmul(
                        ps_l[:, ds(sc, w)],
                        lhsT=ones_sb[:],
                        rhs=ex[:, ds(sc, w)],
                        start=(kt == 0),
                        stop=(kt == n_kt - 1),
                        skip_group_check=True,
                    )
                rec = lpool.tile([1, QB], F32, tag="r", name="rec")
                nc.vector.reciprocal(rec[:], ps_l[:])
                bc = bcpool.tile([128, QB], F32, tag="bc", name="bc")
                nc.gpsimd.partition_broadcast(bc[:], rec[:])
                nc.vector.tensor_mul(
                    ctx_sbs[h][:, ts(qb, QB)], ps_c[:], bc[:]
                )

    # ---------------- Phase 3: output projection ----------------
    with (
        tc.tile_pool(name="o_stage", bufs=4) as ostage,
        tc.tile_pool(name="p3_psum", bufs=4, space="PSUM") as psum3,
    ):
        for m in range(n_sk):
            for n in range(D // QB):
                ps = psum3.tile([128, QB], F32, tag="o", name="ps_p3")
                for k in range(n_dg):
                    nc.tensor.matmul(
                        ps[:],
                        lhsT=ctx_sbs[k][:, ts(m, 128)],
                        rhs=wo_sb[:, ds(k * D + n * QB, QB)],
                        start=(k == 0),
                        stop=(k == n_dg - 1),
                    )
                ot = ostage.tile([128, QB], F32, tag="ot", name="ot")
                nc.vector.tensor_add(ot[:], ps[:], bo_sb[:, ts(n, QB)])
                nc.sync.dma_start(out[ts(m, 128), ts(n, QB)], ot[:])


def build_program(S=SEQ, D=D_MODEL, DGl=DG, enable_asserts=False):
    nc = bacc.Bacc(
        "TRN2",
        target_bir_lowering=False,
        debug=False,
        enable_asserts=enable_asserts,
        num_devices=N_CORES,
    )
    aps = {
        "xt": nc.dram_tensor("xt", [D, S], F32, kind="ExternalInput").ap(),
        "wqt": nc.dram_tensor("wqt", [D, DGl], F32, kind="ExternalInput").ap(),
        "wkt": nc.dram_tensor("wkt", [D, DGl], F32, kind="ExternalInput").ap(),
        "wvt": nc.dram_tensor("wvt", [D, DGl], F32, kind="ExternalInput").ap(),
        "wot": nc.dram_tensor("wot", [DGl, D], F32, kind="ExternalInput").ap(),
        "bq": nc.dram_tensor("bq", [128, DGl // 128], F32, kind="ExternalInput").ap(),
        "bk": nc.dram_tensor("bk", [128, DGl // 128], F32, kind="ExternalInput").ap(),
        "bv": nc.dram_tensor("bv", [128, DGl], F32, kind="ExternalInput").ap(),
        "bo": nc.dram_tensor("bo", [128, D], F32, kind="ExternalInput").ap(),
        "masks": nc.dram_tensor("masks", [4, 128, QB], F32, kind="ExternalInput").ap(),
        "ones": nc.dram_tensor("ones", [128, 1], F32, kind="ExternalInput").ap(),
        "out": nc.dram_tensor("out", [S, D], F32, kind="ExternalOutput").ap(),
    }
    with tile.TileContext(nc) as tc:
        with ExitStack() as ctx:
            _mha_body(ctx, tc, aps, S, D, DGl)
    nc.compile()
    return nc


def make_masks():
    """Additive causal masks: 0 where k<=q, -1e30 where masked."""
    i = np.arange(4)[:, None, None]
    p = np.arange(128)[None, :, None]
    f = np.arange(QB)[None, None, :]
    keep = (i * 128 + p) <= f
    return np.where(keep, 0.0, -1e30).astype(np.float32)


def shard_inputs(x, wq, bq, wk, bk, wv, bv, wo, bo):
    """Build the 8 per-core input maps (host-side layout prep)."""
    masks = make_masks()
    xts = [np.ascontiguousarray(np.asarray(x[b], np.float32).T) for b in range(BATCH)]
    bo_bc = np.ascontiguousarray(
        np.broadcast_to(np.asarray(bo, np.float32), (128, D_MODEL))
    )
    bo_zero = np.zeros((128, D_MODEL), np.float32)
    in_maps = []
    for c in range(N_CORES):
        b, g = divmod(c, N_GROUPS)
        sl = slice(g * DG, (g + 1) * DG)
        in_maps.append(
            {
                "xt": xts[b],
                "wqt": np.ascontiguousarray(np.asarray(wq, np.float32)[sl].T),
                "wkt": np.ascontiguousarray(np.asarray(wk, np.float32)[sl].T),
                "wvt": np.ascontiguousarray(np.asarray(wv, np.float32)[sl].T),
                "wot": np.ascontiguousarray(np.asarray(wo, np.float32)[:, sl].T),
                "bq": np.ascontiguousarray(
                    np.asarray(bq, np.float32)[sl].reshape(-1, 128).T
                ),
                "bk": np.ascontiguousarray(
                    np.asarray(bk, np.float32)[sl].reshape(-1, 128).T
                ),
                "bv": np.ascontiguousarray(
                    np.broadcast_to(np.asarray(bv, np.float32)[sl], (128, DG))
                ),
                "bo": bo_bc if g == 0 else bo_zero,
                "masks": masks,
                "ones": np.ones((128, 1), np.float32),
            }
        )
    return in_maps


_NC_CACHE = {}


def get_program():
    if "nc" not in _NC_CACHE:
        _NC_CACHE["nc"] = build_program()
    return _NC_CACHE["nc"]


def run_sharded(inputs, trace=False):
    nc = get_program()
    in_maps = shard_inputs(**inputs)
    res = run_bass_kernel_spmd(nc, in_maps, list(range(N_CORES)), trace=trace)
    full = np.empty((BATCH, SEQ, D_MODEL), np.float32)
    for b in range(BATCH):
        acc = res.results[b * N_GROUPS]["out"].copy()
        for g in range(1, N_GROUPS):
            acc += res.results[b * N_GROUPS + g]["out"]
        full[b] = acc
    return full, res


def kernel(**inputs):
    out, _ = run_sharded(inputs, trace=False)
    return out



# revision 27
# speedup vs baseline: 1.0455x; 1.0455x over previous
"""Multi-head causal attention block on 8 Trainium2 NeuronCores.

Sharding: tensor-parallel over heads (4 groups of 4 heads) x data-parallel
over batch (2). Core c -> (batch b=c//4, head-group g=c%4). Each core
computes q/k/v projections for its head group, causal attention for its 4
heads, and a partial output projection; the host sums the 4 partials per
batch.

Single fused pipeline: attention (scores bf16, PV / softmax-sum in fp8
DoubleRow), q/k projections (bf16), v projection (fp8 DoubleRow) and the
output projection (bf16) are interleaved per q-block so the exp
(Activation engine) work hides under projection matmuls and the PE never
idles. v stays resident in SBUF (no DRAM round trip); the v bias is folded
into the output bias host-side (wo @ bv is a constant).

Self-contained: hardcodes shapes for the 2x2048x2048, 16-head problem.
"""

from contextlib import ExitStack

import numpy as np
import ml_dtypes

import concourse.bass as bass
import concourse.tile as tile
from concourse import bacc, mybir
from concourse.bass import ds, ts
from concourse.bass_utils import run_bass_kernel_spmd

F32 = mybir.dt.float32
BF16 = mybir.dt.bfloat16
FP8 = mybir.dt.float8e4
ACTF = mybir.ActivationFunctionType
DR = mybir.MatmulPerfMode.DoubleRow

# Full-problem dims
BATCH = 2
SEQ = 2048
D_MODEL = 2048
NUM_HEADS = 16
HEAD_DIM = 128
N_CORES = 8
N_GROUPS = 4  # head-groups (tensor parallel)
DG = D_MODEL // N_GROUPS  # 512 = 4 heads per group
SCALE = 1.0 / float(np.sqrt(HEAD_DIM))

QB = 512  # q-block width (= proj seq-slice width)
N_QB = SEQ // QB  # 4
NKD = D_MODEL // 128  # 16 contraction tiles
NDG = DG // 128  # 4 head tiles per group
NSK = SEQ // 128  # 16 seq subtiles

NP_BF16 = ml_dtypes.bfloat16
NP_FP8 = ml_dtypes.float8_e4m3


def _mha_body(ctx, tc, aps):
    nc = tc.nc
    S, D = SEQ, D_MODEL

    xtb = aps["xtb"]
    wqt, wkt, wvt, wot = aps["wqt"], aps["wkt"], aps["wvt"], aps["wot"]
    out = aps["out"]

    consts = ctx.enter_context(tc.tile_pool(name="consts", bufs=1))
    # dummy Exp first: forces the ACT function-table load to queue ahead of
    # the bulk input loads (else early PSUM drains stall on the table load)
    warm = consts.tile([128, 1], F32, name="act_warm")
    nc.vector.memset(warm[:], 0.0)
    nc.scalar.activation(warm[:], warm[:], ACTF.Exp, scale=1.0)
    zb = consts.tile([128, 1], F32, name="zb")
    nc.vector.memset(zb[:], 0.0)
    ones8 = consts.tile([128, 2, 32], FP8, name="ones8")
    nc.vector.memset(ones8[:], 1.0)
    ones_bf = consts.tile([128, 1], BF16, name="ones_bf")
    nc.vector.memset(ones_bf[:], 1.0)
    bq_sb = consts.tile([128, NDG], F32, name="bq_sb")
    bk_sb = consts.tile([128, NDG], F32, name="bk_sb")
    masks_sb = consts.tile([128, 4 * QB], F32, name="masks_sb")
    bo_sb = consts.tile([128, D], F32, name="bo_sb")

    wpool = ctx.enter_context(tc.tile_pool(name="wpool", bufs=1))
    wq_sb = wpool.tile([128, NKD * DG], BF16, name="wq_sb")
    wk_sb = wpool.tile([128, NKD * DG], BF16, name="wk_sb")
    wv_sb = wpool.tile([128, NKD * DG], BF16, name="wv_sb")
    wo_sb = wpool.tile([128, NDG * D], BF16, name="wo_sb")

    res = ctx.enter_context(tc.tile_pool(name="res", bufs=1))
    kt_res = [res.tile([128, S], BF16, name=f"kt{h}") for h in range(NDG)]
    qt_res = [res.tile([128, S], BF16, name=f"qt{h}") for h in range(NDG)]
    ctx_sb = [res.tile([128, S], BF16, name=f"ctx{h}") for h in range(NDG)]
    v_res = res.tile([128, NSK * DG], FP8, name="v_res")
    # bf16 copy of v rows 0-511: early q-rows average few positions, so the
    # fp8 quantization noise does not cancel there — q-block 0 runs bf16
    v16 = res.tile([128, 4 * DG], BF16, name="v16")

    xb_pool = ctx.enter_context(tc.tile_pool(name="xb", bufs=2))
    ex_pool = ctx.enter_context(tc.tile_pool(name="ex", bufs=4))
    exb_pool = ctx.enter_context(tc.tile_pool(name="exb", bufs=3))
    rec_pool = ctx.enter_context(tc.tile_pool(name="rec", bufs=2))
    bc_pool = ctx.enter_context(tc.tile_pool(name="bc", bufs=2))
    ot_pool = ctx.enter_context(tc.tile_pool(name="ot", bufs=3))

    ps_sc = ctx.enter_context(tc.tile_pool(name="ps_sc", bufs=2, space="PSUM"))
    ps_c = ctx.enter_context(tc.tile_pool(name="ps_c", bufs=1, space="PSUM"))
    ps_l = ctx.enter_context(tc.tile_pool(name="ps_l", bufs=1, space="PSUM"))
    ps_fill = ctx.enter_context(tc.tile_pool(name="ps_fill", bufs=2, space="PSUM"))

    def v3(t2_lo, t2_n, h):
        # [128, t2_n, 128] pair view of v_res for head h
        return v_res[:].rearrange("p (t f) -> p t f", f=DG)[
            :, ds(t2_lo, t2_n), ts(h, 128)
        ]

    # ---------------- DMA loaders (SP/sync queue) ----------------
    def load_w(dst, src, f, nq, n_k=NKD):
        # dst [128, (k f)] <- src [n_k*128, f], in nq k-chunks
        kq = n_k // nq
        for i in range(nq):
            nc.sync.dma_start(
                dst[:, ds(i * kq * f, kq * f)].rearrange("p (k f) -> p k f", k=kq),
                src[ds(i * kq * 128, kq * 128), :].rearrange(
                    "(k p) f -> p k f", p=128
                ),
            )

    def load_x(pool, src, j, dt, nq, tag):
        # [128, (k QB)] slice j of x^T, in nq k-chunks
        t = pool.tile([128, NKD * QB], dt, tag=tag, name=f"{tag}_sb")
        kq = NKD // nq
        for i in range(nq):
            nc.sync.dma_start(
                t[:, ds(i * kq * QB, kq * QB)].rearrange("p (k f) -> p k f", k=kq),
                src[ds(i * kq * 128, kq * 128), ts(j, QB)].rearrange(
                    "(k p) f -> p k f", p=128
                ),
            )
        return t

    # ---------------- work units ----------------
    def emit_kq_unit(proj, j, m, xb_t):
        w_sb, res_t, b_sb = (
            (wk_sb, kt_res, bk_sb) if proj == "k" else (wq_sb, qt_res, bq_sb)
        )
        ps = ps_fill.tile([128, QB], F32, tag="fill", name="ps_fill")
        for k in range(NKD):
            nc.tensor.matmul(
                ps[:],
                lhsT=w_sb[:, ds(k * DG + m * 128, 128)],
                rhs=xb_t[:, ts(k, QB)],
                start=(k == 0),
                stop=(k == NKD - 1),
            )
        nc.scalar.activation(
            res_t[m][:, ts(j, QB)], ps[:], ACTF.Identity, bias=b_sb[:, ds(m, 1)]
        )

    def emit_v_unit(t, xb_t, s_local):
        ps = ps_fill.tile([128, DG], F32, tag="fill", name="ps_fill")
        for k in range(NKD):
            nc.tensor.matmul(
                ps[:],
                lhsT=xb_t[:, ds(k * QB + s_local * 128, 128)],
                rhs=wv_sb[:, ts(k, DG)],
                start=(k == 0),
                stop=(k == NKD - 1),
            )
        nc.scalar.activation(
            v_res[:, ds(t * DG, DG)], ps[:], ACTF.Identity, bias=zb[:, 0:1]
        )
        if t < 4:
            nc.scalar.activation(
                v16[:, ds(t * DG, DG)], ps[:], ACTF.Identity, bias=zb[:, 0:1]
            )

    def emit_out_unit(m, n):
        ps = ps_fill.tile([128, QB], F32, tag="fill", name="ps_fill")
        for h in range(NDG):
            nc.tensor.matmul(
                ps[:],
                lhsT=ctx_sb[h][:, ts(m, 128)],
                rhs=wo_sb[:, ds(h * D + n * QB, QB)],
                start=(h == 0),
                stop=(h == NDG - 1),
            )
        ot = ot_pool.tile([128, QB], F32, tag="ot", name="ot")
        nc.vector.tensor_add(ot[:], ps[:], bo_sb[:, ts(n, QB)])
        nc.gpsimd.dma_start(out[ts(m, 128), ts(n, QB)], ot[:])

    # filler queue: (deadline_stage, est_ns, closure). Units with
    # deadline <= current stage are force-emitted at stage start (their
    # results are read by that stage's attention); the rest pace the rounds.
    filler = []

    def push_slice_units(proj, j, xb_t):
        for m in range(NDG):
            filler.append(
                (j, 3413, lambda p=proj, jj=j, mm=m, t=xb_t: emit_kq_unit(p, jj, mm, t))
            )

    def push_v_units(jb, xb_t):
        for s in range(QB // 128):
            filler.append(
                (jb, 3413, lambda t=jb * 4 + s, xt=xb_t, sl=s: emit_v_unit(t, xt, sl))
            )

    def push_out_units(qb):
        for m in range(qb * 4, qb * 4 + 4):
            for n in range(N_QB):
                filler.append((99, 853, lambda mm=m, nn=n: emit_out_unit(mm, nn)))

    def emit_due(stage):
        rest = []
        for d, est, fn in filler:
            if d <= stage:
                fn()
            else:
                rest.append((d, est, fn))
        filler[:] = rest

    def emit_filler(budget):
        while budget > 0 and filler:
            _, est, fn = filler.pop(0)
            fn()
            budget -= est

    MASK_RANGES = {0: (0, 128), 1: (0, 256), 2: (256, 384), 3: (256, 512)}

    # ---------------- attention for one (qb, head) ----------------
    def emit_attn_head(qb, h, budget_per_round):
        n_pairs = 2 * (qb + 1)
        n_kt = 4 * (qb + 1)
        pc = ps_c.tile([128, QB], F32, tag=f"c{h % 2}", name="ps_c")
        pl = ps_l.tile([32, QB], F32, tag=f"l{h % 2}", name="ps_l")
        ex_tiles = {}

        def pair_sc(t2):
            # last pair of the q-block covers diagonal offs (2,3): cols 256+
            return 256 if t2 == n_pairs - 1 else 0

        def emit_scores(t2):
            sc = pair_sc(t2)
            w = QB - sc
            if qb == 0:
                ex = exb_pool.tile([128, 2 * QB], BF16, tag="exb", name="exb")
            else:
                ex = ex_pool.tile([128, 2 * QB], FP8, tag="ex", name="ex")
            for i in range(2):
                kt = 2 * t2 + i
                off = kt - (n_kt - 4)
                ps = ps_sc.tile([128, QB], F32, tag="sc", name="ps_sc")
                nc.tensor.matmul(
                    ps[:, ds(sc, w)],
                    lhsT=kt_res[h][:, ts(kt, 128)],
                    rhs=qt_res[h][:, ds(qb * QB + sc, w)],
                    start=True,
                    stop=True,
                )
                if off >= 0:
                    lo, hi = MASK_RANGES[off]
                    nc.vector.tensor_add(
                        ps[:, ds(lo, hi - lo)],
                        ps[:, ds(lo, hi - lo)],
                        masks_sb[:, ds(off * QB + lo, hi - lo)],
                    )
                nc.scalar.activation(
                    ex[:, ds(i * QB + sc, w)],
                    ps[:, ds(sc, w)],
                    ACTF.Exp,
                    scale=SCALE,
                )
            ex_tiles[t2] = ex

        def emit_pv_l(t2):
            sc = pair_sc(t2)
            w = QB - sc
            ex = ex_tiles.pop(t2)
            if qb == 0:
                # bf16 path: per-128 k-subtile matmuls against the bf16 v copy
                for i in range(2):
                    kt = 2 * t2 + i
                    nc.tensor.matmul(
                        pc[:, ds(sc, w)],
                        lhsT=v16[:, ds(kt * DG + h * 128, 128)],
                        rhs=ex[:, ds(i * QB + sc, w)],
                        start=(kt == 0),
                        stop=(kt == 2 * n_pairs - 1),
                        skip_group_check=True,
                    )
                    nc.tensor.matmul(
                        pl[ds(0, 1), ds(sc, w)],
                        lhsT=ones_bf[:],
                        rhs=ex[:, ds(i * QB + sc, w)],
                        start=(kt == 0),
                        stop=(kt == 2 * n_pairs - 1),
                        skip_group_check=True,
                    )
                return
            exv = ex[:].rearrange("p (two f) -> p two f", two=2)[:, :, ds(sc, w)]
            nc.tensor.matmul(
                pc[:, ds(sc, w)],
                lhsT=v3(2 * t2, 2, h),
                rhs=exv,
                start=(t2 == 0),
                stop=(t2 == n_pairs - 1),
                perf_mode=DR,
                skip_group_check=True,
            )
            nc.tensor.matmul(
                pl[:, ds(sc, w)],
                lhsT=ones8[:],
                rhs=exv,
                start=(t2 == 0),
                stop=(t2 == n_pairs - 1),
                perf_mode=DR,
                skip_group_check=True,
            )

        for t2 in range(n_pairs):
            emit_scores(t2)
            emit_filler(budget_per_round)
            emit_pv_l(t2)
        rec = rec_pool.tile([1, QB], F32, tag="r", name="rec")
        nc.vector.reciprocal(rec[:], pl[ds(0, 1), :])
        bc = bc_pool.tile([128, QB], F32, tag="bc", name="bc")
        nc.gpsimd.partition_broadcast(bc[:], rec[:])
        nc.vector.tensor_mul(ctx_sb[h][:, ts(qb, QB)], pc[:], bc[:])

    # ---------------- prologue DMAs ----------------
    nc.sync.dma_start(bq_sb[:], aps["bq"])
    nc.sync.dma_start(bk_sb[:], aps["bk"])
    # interleave wk / x slice-0 quarters so the first k-proj unit starts early
    xb_t = {}
    xb_t[0] = xb_pool.tile([128, NKD * QB], BF16, tag="xb", name="xb_sb")
    for i in range(4):
        nc.sync.dma_start(
            wk_sb[:, ds(i * 4 * DG, 4 * DG)].rearrange("p (k f) -> p k f", k=4),
            wkt[ds(i * 4 * 128, 4 * 128), :].rearrange("(k p) f -> p k f", p=128),
        )
        nc.sync.dma_start(
            xb_t[0][:, ds(i * 4 * QB, 4 * QB)].rearrange("p (k f) -> p k f", k=4),
            xtb[ds(i * 4 * 128, 4 * 128), 0:QB].rearrange("(k p) f -> p k f", p=128),
        )
    load_w(wq_sb, wqt, DG, 2)
    load_w(wv_sb, wvt, DG, 2)
    xb_t[1] = load_x(xb_pool, xtb, 1, BF16, 2, "xb")
    nc.sync.dma_start(
        masks_sb[:].rearrange("p (i f) -> p i f", i=4),
        aps["masks"].rearrange("i p f -> p i f"),
    )
    nc.sync.dma_start(bo_sb[:], aps["bo"])
    load_w(wo_sb, wot, D, 1, n_k=NDG)

    # ---------------- prologue compute: slice 0 proj + v block 0 ----------
    for m in range(NDG):
        emit_kq_unit("k", 0, m, xb_t[0])
    for m in range(NDG):
        emit_kq_unit("q", 0, m, xb_t[0])
    for s in range(4):
        emit_v_unit(s, xb_t[0], s)

    # ---------------- main interleaved schedule ----------------
    total_rounds = sum(2 * (qb + 1) * NDG for qb in range(N_QB))  # 80
    rounds_left = total_rounds

    def budget():
        est = sum(e for _, e, _ in filler)
        return max(1200, est // max(1, rounds_left))

    for qb in range(N_QB):
        if qb + 1 < N_QB:
            if qb + 1 not in xb_t:
                xb_t[qb + 1] = load_x(xb_pool, xtb, qb + 1, BF16, 2, "xb")
            push_slice_units("k", qb + 1, xb_t[qb + 1])
            push_slice_units("q", qb + 1, xb_t[qb + 1])
            push_v_units(qb + 1, xb_t[qb + 1])
        emit_due(qb)  # everything this stage's attention reads must be emitted
        for h in range(NDG):
            emit_attn_head(qb, h, budget())
            rounds_left -= 2 * (qb + 1)
        push_out_units(qb)

    # epilogue: drain remaining filler (last out-proj tiles)
    emit_filler(1 << 60)


def build_program(enable_asserts=False):
    nc = bacc.Bacc(
        "TRN2",
        target_bir_lowering=False,
        debug=False,
        enable_asserts=enable_asserts,
        num_devices=N_CORES,
    )
    aps = {
        "xtb": nc.dram_tensor("xtb", [D_MODEL, SEQ], BF16, kind="ExternalInput").ap(),
        "wqt": nc.dram_tensor("wqt", [D_MODEL, DG], BF16, kind="ExternalInput").ap(),
        "wkt": nc.dram_tensor("wkt", [D_MODEL, DG], BF16, kind="ExternalInput").ap(),
        "wvt": nc.dram_tensor("wvt", [D_MODEL, DG], BF16, kind="ExternalInput").ap(),
        "wot": nc.dram_tensor("wot", [DG, D_MODEL], BF16, kind="ExternalInput").ap(),
        "bq": nc.dram_tensor("bq", [128, NDG], F32, kind="ExternalInput").ap(),
        "bk": nc.dram_tensor("bk", [128, NDG], F32, kind="ExternalInput").ap(),
        "bo": nc.dram_tensor("bo", [128, D_MODEL], F32, kind="ExternalInput").ap(),
        "masks": nc.dram_tensor("masks", [4, 128, QB], F32, kind="ExternalInput").ap(),
        "out": nc.dram_tensor("out", [SEQ, D_MODEL], F32, kind="ExternalOutput").ap(),
    }
    with tile.TileContext(nc) as tc:
        with ExitStack() as ctx:
            _mha_body(ctx, tc, aps)
    nc.compile()
    return nc


def make_masks():
    """Additive causal masks: 0 where k<=q, -1e30 where masked."""
    i = np.arange(4)[:, None, None]
    p = np.arange(128)[None, :, None]
    f = np.arange(QB)[None, None, :]
    keep = (i * 128 + p) <= f
    return np.where(keep, 0.0, -1e30).astype(np.float32)


def shard_inputs(x, wq, bq, wk, bk, wv, bv, wo, bo):
    """Build the 8 per-core input maps (host-side layout + dtype prep)."""
    masks = make_masks()
    x = np.asarray(x, np.float32)
    wq, wk, wv, wo = (np.asarray(a, np.float32) for a in (wq, wk, wv, wo))
    bq, bk, bv, bo = (np.asarray(a, np.float32) for a in (bq, bk, bv, bo))
    xts = [np.ascontiguousarray(x[b].T) for b in range(BATCH)]
    xtbs = [t.astype(NP_BF16) for t in xts]
    in_maps = []
    for c in range(N_CORES):
        b, g = divmod(c, N_GROUPS)
        sl = slice(g * DG, (g + 1) * DG)
        # fold the v bias through the output projection: out += wo[:,sl] @ bv[sl]
        bo_eff = wo[:, sl].astype(np.float64) @ bv[sl].astype(np.float64)
        if g == 0:
            bo_eff = bo_eff + bo
        bo_bc = np.ascontiguousarray(
            np.broadcast_to(bo_eff.astype(np.float32), (128, D_MODEL))
        )
        in_maps.append(
            {
                "xtb": xtbs[b],
                "wqt": np.ascontiguousarray(wq[sl].T).astype(NP_BF16),
                "wkt": np.ascontiguousarray(wk[sl].T).astype(NP_BF16),
                "wvt": np.ascontiguousarray(wv[sl].T).astype(NP_BF16),
                "wot": np.ascontiguousarray(wo[:, sl].T).astype(NP_BF16),
                "bq": np.ascontiguousarray(bq[sl].reshape(-1, 128).T),
                "bk": np.ascontiguousarray(bk[sl].reshape(-1, 128).T),
                "bo": bo_bc,
                "masks": masks,
            }
        )
    return in_maps


_NC_CACHE = {}


def get_program():
    if "nc" not in _NC_CACHE:
        _NC_CACHE["nc"] = build_program()
    return _NC_CACHE["nc"]


def run_sharded(inputs, trace=False):
    nc = get_program()
    in_maps = shard_inputs(**inputs)
    res = run_bass_kernel_spmd(nc, in_maps, list(range(N_CORES)), trace=trace)
    full = np.empty((BATCH, SEQ, D_MODEL), np.float32)
    for b in range(BATCH):
        acc = res.results[b * N_GROUPS]["out"].copy()
        for g in range(1, N_GROUPS):
            acc += res.results[b * N_GROUPS + g]["out"]
        full[b] = acc
    return full, res


def kernel(**inputs):
    out, _ = run_sharded(inputs, trace=False)
    return out


# revision 31
# speedup vs baseline: 1.0950x; 1.0473x over previous
"""Multi-head causal attention block on 8 Trainium2 NeuronCores.

Sharding: tensor-parallel over heads (4 groups of 4 heads) x data-parallel
over batch (2). Core c -> (batch b=c//4, head-group g=c%4). Each core
computes q/k/v projections for its head group, causal attention for its 4
heads, and a partial output projection; the host sums the 4 partials per
batch.

Single fused pipeline: attention (scores bf16, PV / softmax-sum in fp8
DoubleRow), q/k projections (bf16), v projection (fp8 DoubleRow) and the
output projection (bf16) are interleaved per q-block so the exp
(Activation engine) work hides under projection matmuls and the PE never
idles. v stays resident in SBUF (no DRAM round trip); the v bias is folded
into the output bias host-side (wo @ bv is a constant).

Self-contained: hardcodes shapes for the 2x2048x2048, 16-head problem.
"""

from contextlib import ExitStack

import numpy as np
import ml_dtypes

import concourse.bass as bass
import concourse.tile as tile
from concourse import bacc, mybir
from concourse.bass import ds, ts
from concourse.bass_utils import run_bass_kernel_spmd

F32 = mybir.dt.float32
BF16 = mybir.dt.bfloat16
FP8 = mybir.dt.float8e4
ACTF = mybir.ActivationFunctionType
DR = mybir.MatmulPerfMode.DoubleRow

# Full-problem dims
BATCH = 2
SEQ = 2048
D_MODEL = 2048
NUM_HEADS = 16
HEAD_DIM = 128
N_CORES = 8
N_GROUPS = 4  # head-groups (tensor parallel)
DG = D_MODEL // N_GROUPS  # 512 = 4 heads per group
SCALE = 1.0 / float(np.sqrt(HEAD_DIM))

QB = 512  # q-block width (= proj seq-slice width)
N_QB = SEQ // QB  # 4
NKD = D_MODEL // 128  # 16 contraction tiles
NDG = DG // 128  # 4 head tiles per group
NSK = SEQ // 128  # 16 seq subtiles

NP_BF16 = ml_dtypes.bfloat16
NP_FP8 = ml_dtypes.float8_e4m3


def _mha_body(ctx, tc, aps):
    nc = tc.nc
    S, D = SEQ, D_MODEL

    xtb = aps["xtb"]
    wqt, wkt, wvt, wot = aps["wqt"], aps["wkt"], aps["wvt"], aps["wot"]
    out = aps["out"]

    consts = ctx.enter_context(tc.tile_pool(name="consts", bufs=1))
    # dummy Exp first: forces the ACT function-table load to queue ahead of
    # the bulk input loads (else early PSUM drains stall on the table load)
    warm = consts.tile([128, 1], F32, name="act_warm")
    nc.vector.memset(warm[:], 0.0)
    nc.scalar.activation(warm[:], warm[:], ACTF.Exp, scale=1.0)
    zb = consts.tile([128, 1], F32, name="zb")
    nc.vector.memset(zb[:], 0.0)
    ones8 = consts.tile([128, 2, 32], FP8, name="ones8")
    nc.vector.memset(ones8[:], 1.0)
    ones_bf = consts.tile([128, 1], BF16, name="ones_bf")
    nc.vector.memset(ones_bf[:], 1.0)
    bq_sb = consts.tile([128, NDG], F32, name="bq_sb")
    bk_sb = consts.tile([128, NDG], F32, name="bk_sb")
    masks_sb = consts.tile([128, 4 * QB], F32, name="masks_sb")
    bo_sb = consts.tile([128, D], F32, name="bo_sb")

    wpool = ctx.enter_context(tc.tile_pool(name="wpool", bufs=1))
    wq_sb = wpool.tile([128, NKD * DG], BF16, name="wq_sb")
    wk_sb = wpool.tile([128, NKD * DG], BF16, name="wk_sb")
    wv_sb = wpool.tile([128, NKD * DG], BF16, name="wv_sb")
    wo_sb = wpool.tile([128, NDG * D], BF16, name="wo_sb")

    res = ctx.enter_context(tc.tile_pool(name="res", bufs=1))
    kt_res = [res.tile([128, S], BF16, name=f"kt{h}") for h in range(NDG)]
    qt_res = [res.tile([128, S], BF16, name=f"qt{h}") for h in range(NDG)]
    ctx_sb = [res.tile([128, S], BF16, name=f"ctx{h}") for h in range(NDG)]
    v_res = res.tile([128, NSK * DG], FP8, name="v_res")
    # bf16 copy of v rows 0-511: early q-rows average few positions, so the
    # fp8 quantization noise does not cancel there — q-block 0 runs bf16
    v16 = res.tile([128, 4 * DG], BF16, name="v16")

    xb_pool = ctx.enter_context(tc.tile_pool(name="xb", bufs=2))
    ex_pool = ctx.enter_context(tc.tile_pool(name="ex", bufs=4))
    exb_pool = ctx.enter_context(tc.tile_pool(name="exb", bufs=3))
    rec_pool = ctx.enter_context(tc.tile_pool(name="rec", bufs=2))
    bc_pool = ctx.enter_context(tc.tile_pool(name="bc", bufs=2))
    ot_pool = ctx.enter_context(tc.tile_pool(name="ot", bufs=3))

    ps_sc = ctx.enter_context(tc.tile_pool(name="ps_sc", bufs=2, space="PSUM"))
    ps_c = ctx.enter_context(tc.tile_pool(name="ps_c", bufs=1, space="PSUM"))
    ps_l = ctx.enter_context(tc.tile_pool(name="ps_l", bufs=1, space="PSUM"))
    ps_fill = ctx.enter_context(tc.tile_pool(name="ps_fill", bufs=2, space="PSUM"))

    def v3(t2_lo, t2_n, h):
        # [128, t2_n, 128] pair view of v_res for head h
        return v_res[:].rearrange("p (t f) -> p t f", f=DG)[
            :, ds(t2_lo, t2_n), ts(h, 128)
        ]

    # ---------------- DMA loaders (SP/sync queue) ----------------
    def load_w(dst, src, f, nq, n_k=NKD):
        # dst [128, (k f)] <- src [n_k*128, f], in nq k-chunks
        kq = n_k // nq
        for i in range(nq):
            nc.sync.dma_start(
                dst[:, ds(i * kq * f, kq * f)].rearrange("p (k f) -> p k f", k=kq),
                src[ds(i * kq * 128, kq * 128), :].rearrange(
                    "(k p) f -> p k f", p=128
                ),
            )

    def load_x(pool, src, j, dt, nq, tag):
        # [128, (k QB)] slice j of x^T, in nq k-chunks
        t = pool.tile([128, NKD * QB], dt, tag=tag, name=f"{tag}_sb")
        kq = NKD // nq
        for i in range(nq):
            nc.sync.dma_start(
                t[:, ds(i * kq * QB, kq * QB)].rearrange("p (k f) -> p k f", k=kq),
                src[ds(i * kq * 128, kq * 128), ts(j, QB)].rearrange(
                    "(k p) f -> p k f", p=128
                ),
            )
        return t

    # ---------------- work units ----------------
    def emit_kq_unit(proj, j, m, xb_t):
        w_sb, res_t, b_sb = (
            (wk_sb, kt_res, bk_sb) if proj == "k" else (wq_sb, qt_res, bq_sb)
        )
        ps = ps_fill.tile([128, QB], F32, tag="fill", name="ps_fill")
        for k in range(NKD):
            nc.tensor.matmul(
                ps[:],
                lhsT=w_sb[:, ds(k * DG + m * 128, 128)],
                rhs=xb_t[:, ts(k, QB)],
                start=(k == 0),
                stop=(k == NKD - 1),
            )
        # bias-add drain on DVE keeps the Activation engine free for exps
        nc.vector.tensor_scalar_add(
            res_t[m][:, ts(j, QB)], ps[:], b_sb[:, ds(m, 1)]
        )

    def emit_v_unit(t, xb_t, s_local):
        ps = ps_fill.tile([128, DG], F32, tag="fill", name="ps_fill")
        for k in range(NKD):
            nc.tensor.matmul(
                ps[:],
                lhsT=xb_t[:, ds(k * QB + s_local * 128, 128)],
                rhs=wv_sb[:, ts(k, DG)],
                start=(k == 0),
                stop=(k == NKD - 1),
            )
        nc.scalar.activation(
            v_res[:, ds(t * DG, DG)], ps[:], ACTF.Identity, bias=zb[:, 0:1]
        )
        if t < 4:
            nc.scalar.activation(
                v16[:, ds(t * DG, DG)], ps[:], ACTF.Identity, bias=zb[:, 0:1]
            )

    def emit_out_unit(m, n):
        ps = ps_fill.tile([128, QB], F32, tag="fill", name="ps_fill")
        for h in range(NDG):
            nc.tensor.matmul(
                ps[:],
                lhsT=ctx_sb[h][:, ts(m, 128)],
                rhs=wo_sb[:, ds(h * D + n * QB, QB)],
                start=(h == 0),
                stop=(h == NDG - 1),
            )
        ot = ot_pool.tile([128, QB], F32, tag="ot", name="ot")
        nc.vector.tensor_add(ot[:], ps[:], bo_sb[:, ts(n, QB)])
        nc.gpsimd.dma_start(out[ts(m, 128), ts(n, QB)], ot[:])

    # filler queue: (deadline_stage, est_ns, closure). Units with
    # deadline <= current stage are force-emitted at stage start (their
    # results are read by that stage's attention); the rest pace the rounds.
    filler = []

    def push_slice_units(proj, j, xb_t):
        for m in range(NDG):
            filler.append(
                (j, 3413, lambda p=proj, jj=j, mm=m, t=xb_t: emit_kq_unit(p, jj, mm, t))
            )

    def push_v_units(jb, xb_t):
        for s in range(QB // 128):
            filler.append(
                (jb, 3413, lambda t=jb * 4 + s, xt=xb_t, sl=s: emit_v_unit(t, xt, sl))
            )

    def push_out_units(qb):
        for m in range(qb * 4, qb * 4 + 4):
            for n in range(N_QB):
                filler.append((99, 853, lambda mm=m, nn=n: emit_out_unit(mm, nn)))

    def emit_due(stage):
        rest = []
        for d, est, fn in filler:
            if d <= stage:
                fn()
            else:
                rest.append((d, est, fn))
        filler[:] = rest

    # Round pacing: each attention round accrues an allowance of
    # (remaining filler) / (remaining rounds) and pops units while they fit,
    # carrying any remainder so big units never overshoot the round budget.
    sched = {"allowance": 0.0, "rounds_left": 1}

    def emit_filler(_budget=None):
        if not filler:
            return
        est_total = sum(e for _, e, _ in filler)
        sched["allowance"] += est_total / max(1, sched["rounds_left"])
        while filler and filler[0][1] <= sched["allowance"]:
            _, est, fn = filler.pop(0)
            fn()
            sched["allowance"] -= est
        sched["allowance"] = min(sched["allowance"], 4000.0)

    MASK_RANGES = {0: (0, 128), 1: (0, 256), 2: (256, 384), 3: (256, 512)}

    # ---------------- attention for one (qb, head) ----------------
    def emit_attn_head(qb, h, budget_per_round):
        n_pairs = 2 * (qb + 1)
        n_kt = 4 * (qb + 1)
        pc = ps_c.tile([128, QB], F32, tag=f"c{h % 2}", name="ps_c")
        pl = ps_l.tile([32, QB], F32, tag=f"l{h % 2}", name="ps_l")
        ex_tiles = {}

        def pair_sc(t2):
            # last pair of the q-block covers diagonal offs (2,3): cols 256+
            return 256 if t2 == n_pairs - 1 else 0

        def emit_scores(t2):
            sc = pair_sc(t2)
            w = QB - sc
            if qb == 0:
                ex = exb_pool.tile([128, 2 * QB], BF16, tag="exb", name="exb")
            else:
                ex = ex_pool.tile([128, 2 * QB], FP8, tag="ex", name="ex")
            for i in range(2):
                kt = 2 * t2 + i
                off = kt - (n_kt - 4)
                ps = ps_sc.tile([128, QB], F32, tag="sc", name="ps_sc")
                nc.tensor.matmul(
                    ps[:, ds(sc, w)],
                    lhsT=kt_res[h][:, ts(kt, 128)],
                    rhs=qt_res[h][:, ds(qb * QB + sc, w)],
                    start=True,
                    stop=True,
                )
                if off >= 0:
                    lo, hi = MASK_RANGES[off]
                    nc.vector.tensor_add(
                        ps[:, ds(lo, hi - lo)],
                        ps[:, ds(lo, hi - lo)],
                        masks_sb[:, ds(off * QB + lo, hi - lo)],
                    )
                nc.scalar.activation(
                    ex[:, ds(i * QB + sc, w)],
                    ps[:, ds(sc, w)],
                    ACTF.Exp,
                    scale=SCALE,
                )
            ex_tiles[t2] = ex

        def emit_pv_l(t2):
            sc = pair_sc(t2)
            w = QB - sc
            ex = ex_tiles.pop(t2)
            if qb == 0:
                # bf16 path: per-128 k-subtile matmuls against the bf16 v copy
                for i in range(2):
                    kt = 2 * t2 + i
                    nc.tensor.matmul(
                        pc[:, ds(sc, w)],
                        lhsT=v16[:, ds(kt * DG + h * 128, 128)],
                        rhs=ex[:, ds(i * QB + sc, w)],
                        start=(kt == 0),
                        stop=(kt == 2 * n_pairs - 1),
                        skip_group_check=True,
                    )
                    nc.tensor.matmul(
                        pl[ds(0, 1), ds(sc, w)],
                        lhsT=ones_bf[:],
                        rhs=ex[:, ds(i * QB + sc, w)],
                        start=(kt == 0),
                        stop=(kt == 2 * n_pairs - 1),
                        skip_group_check=True,
                    )
                return
            exv = ex[:].rearrange("p (two f) -> p two f", two=2)[:, :, ds(sc, w)]
            nc.tensor.matmul(
                pc[:, ds(sc, w)],
                lhsT=v3(2 * t2, 2, h),
                rhs=exv,
                start=(t2 == 0),
                stop=(t2 == n_pairs - 1),
                perf_mode=DR,
                skip_group_check=True,
            )
            nc.tensor.matmul(
                pl[:, ds(sc, w)],
                lhsT=ones8[:],
                rhs=exv,
                start=(t2 == 0),
                stop=(t2 == n_pairs - 1),
                perf_mode=DR,
                skip_group_check=True,
            )

        for t2 in range(n_pairs):
            emit_scores(t2)
            emit_filler()
            emit_pv_l(t2)
            sched["rounds_left"] -= 1
        rec = rec_pool.tile([1, QB], F32, tag="r", name="rec")
        nc.vector.reciprocal(rec[:], pl[ds(0, 1), :])
        bc = bc_pool.tile([128, QB], F32, tag="bc", name="bc")
        nc.gpsimd.partition_broadcast(bc[:], rec[:])
        nc.vector.tensor_mul(ctx_sb[h][:, ts(qb, QB)], pc[:], bc[:])

    # ---------------- prologue DMAs ----------------
    nc.sync.dma_start(bq_sb[:], aps["bq"])
    nc.sync.dma_start(bk_sb[:], aps["bk"])
    # interleave wk / x slice-0 quarters so the first k-proj unit starts early
    xb_t = {}
    xb_t[0] = xb_pool.tile([128, NKD * QB], BF16, tag="xb", name="xb_sb")
    for i in range(4):
        nc.sync.dma_start(
            wk_sb[:, ds(i * 4 * DG, 4 * DG)].rearrange("p (k f) -> p k f", k=4),
            wkt[ds(i * 4 * 128, 4 * 128), :].rearrange("(k p) f -> p k f", p=128),
        )
        nc.sync.dma_start(
            xb_t[0][:, ds(i * 4 * QB, 4 * QB)].rearrange("p (k f) -> p k f", k=4),
            xtb[ds(i * 4 * 128, 4 * 128), 0:QB].rearrange("(k p) f -> p k f", p=128),
        )
    load_w(wq_sb, wqt, DG, 2)
    load_w(wv_sb, wvt, DG, 2)
    xb_t[1] = load_x(xb_pool, xtb, 1, BF16, 2, "xb")
    nc.sync.dma_start(
        masks_sb[:].rearrange("p (i f) -> p i f", i=4),
        aps["masks"].rearrange("i p f -> p i f"),
    )
    nc.sync.dma_start(bo_sb[:], aps["bo"])
    load_w(wo_sb, wot, D, 1, n_k=NDG)

    # ---------------- prologue compute: slice 0 proj + v block 0 ----------
    for m in range(NDG):
        emit_kq_unit("k", 0, m, xb_t[0])
    for m in range(NDG):
        emit_kq_unit("q", 0, m, xb_t[0])
    for s in range(4):
        emit_v_unit(s, xb_t[0], s)

    # ---------------- main interleaved schedule ----------------
    total_rounds = sum(2 * (qb + 1) * NDG for qb in range(N_QB))  # 80
    sched["rounds_left"] = total_rounds

    for qb in range(N_QB):
        if qb + 1 < N_QB:
            if qb + 1 not in xb_t:
                xb_t[qb + 1] = load_x(xb_pool, xtb, qb + 1, BF16, 2, "xb")
            push_slice_units("k", qb + 1, xb_t[qb + 1])
            push_slice_units("q", qb + 1, xb_t[qb + 1])
            push_v_units(qb + 1, xb_t[qb + 1])
        emit_due(qb)  # everything this stage's attention reads must be emitted
        for h in range(NDG):
            emit_attn_head(qb, h, None)
        push_out_units(qb)

    # epilogue: drain remaining filler (last out-proj tiles)
    emit_filler(1 << 60)


def build_program(enable_asserts=False):
    nc = bacc.Bacc(
        "TRN2",
        target_bir_lowering=False,
        debug=False,
        enable_asserts=enable_asserts,
        num_devices=N_CORES,
    )
    aps = {
        "xtb": nc.dram_tensor("xtb", [D_MODEL, SEQ], BF16, kind="ExternalInput").ap(),
        "wqt": nc.dram_tensor("wqt", [D_MODEL, DG], BF16, kind="ExternalInput").ap(),
        "wkt": nc.dram_tensor("wkt", [D_MODEL, DG], BF16, kind="ExternalInput").ap(),
        "wvt": nc.dram_tensor("wvt", [D_MODEL, DG], BF16, kind="ExternalInput").ap(),
        "wot": nc.dram_tensor("wot", [DG, D_MODEL], BF16, kind="ExternalInput").ap(),
        "bq": nc.dram_tensor("bq", [128, NDG], F32, kind="ExternalInput").ap(),
        "bk": nc.dram_tensor("bk", [128, NDG], F32, kind="ExternalInput").ap(),
        "bo": nc.dram_tensor("bo", [128, D_MODEL], F32, kind="ExternalInput").ap(),
        "masks": nc.dram_tensor("masks", [4, 128, QB], F32, kind="ExternalInput").ap(),
        "out": nc.dram_tensor("out", [SEQ, D_MODEL], F32, kind="ExternalOutput").ap(),
    }
    with tile.TileContext(nc) as tc:
        with ExitStack() as ctx:
            _mha_body(ctx, tc, aps)
    nc.compile()
    return nc


def make_masks():
    """Additive causal masks: 0 where k<=q, -1e30 where masked."""
    i = np.arange(4)[:, None, None]
    p = np.arange(128)[None, :, None]
    f = np.arange(QB)[None, None, :]
    keep = (i * 128 + p) <= f
    return np.where(keep, 0.0, -1e30).astype(np.float32)


def shard_inputs(x, wq, bq, wk, bk, wv, bv, wo, bo):
    """Build the 8 per-core input maps (host-side layout + dtype prep)."""
    masks = make_masks()
    x = np.asarray(x, np.float32)
    wq, wk, wv, wo = (np.asarray(a, np.float32) for a in (wq, wk, wv, wo))
    bq, bk, bv, bo = (np.asarray(a, np.float32) for a in (bq, bk, bv, bo))
    xts = [np.ascontiguousarray(x[b].T) for b in range(BATCH)]
    xtbs = [t.astype(NP_BF16) for t in xts]
    in_maps = []
    for c in range(N_CORES):
        b, g = divmod(c, N_GROUPS)
        sl = slice(g * DG, (g + 1) * DG)
        # fold the v bias through the output projection: out += wo[:,sl] @ bv[sl]
        bo_eff = wo[:, sl].astype(np.float64) @ bv[sl].astype(np.float64)
        if g == 0:
            bo_eff = bo_eff + bo
        bo_bc = np.ascontiguousarray(
            np.broadcast_to(bo_eff.astype(np.float32), (128, D_MODEL))
        )
        in_maps.append(
            {
                "xtb": xtbs[b],
                "wqt": np.ascontiguousarray(wq[sl].T).astype(NP_BF16),
                "wkt": np.ascontiguousarray(wk[sl].T).astype(NP_BF16),
                "wvt": np.ascontiguousarray(wv[sl].T).astype(NP_BF16),
                "wot": np.ascontiguousarray(wo[:, sl].T).astype(NP_BF16),
                "bq": np.ascontiguousarray(bq[sl].reshape(-1, 128).T),
                "bk": np.ascontiguousarray(bk[sl].reshape(-1, 128).T),
                "bo": bo_bc,
                "masks": masks,
            }
        )
    return in_maps


_NC_CACHE = {}


def get_program():
    if "nc" not in _NC_CACHE:
        _NC_CACHE["nc"] = build_program()
    return _NC_CACHE["nc"]


def run_sharded(inputs, trace=False):
    nc = get_program()
    in_maps = shard_inputs(**inputs)
    res = run_bass_kernel_spmd(nc, in_maps, list(range(N_CORES)), trace=trace)
    full = np.empty((BATCH, SEQ, D_MODEL), np.float32)
    for b in range(BATCH):
        acc = res.results[b * N_GROUPS]["out"].copy()
        for g in range(1, N_GROUPS):
            acc += res.results[b * N_GROUPS + g]["out"]
        full[b] = acc
    return full, res


def kernel(**inputs):
    out, _ = run_sharded(inputs, trace=False)
    return out


# revision 39
# speedup vs baseline: 1.1324x; 1.0341x over previous
"""Multi-head causal attention block on 8 Trainium2 NeuronCores.

Sharding: tensor-parallel over heads (4 groups of 4 heads) x data-parallel
over batch (2). Core c -> (batch b=c//4, head-group g=c%4). Each core
computes q/k/v projections for its head group, causal attention for its 4
heads, and a partial output projection; the host sums the 4 partials per
batch.

Single fused pipeline: attention (scores bf16, PV / softmax-sum in fp8
DoubleRow), q/k projections (bf16), v projection (fp8 DoubleRow) and the
output projection (bf16) are interleaved per q-block so the exp
(Activation engine) work hides under projection matmuls and the PE never
idles. v stays resident in SBUF (no DRAM round trip); the v bias is folded
into the output bias host-side (wo @ bv is a constant).

Self-contained: hardcodes shapes for the 2x2048x2048, 16-head problem.
"""

from contextlib import ExitStack

import numpy as np
import ml_dtypes

import concourse.bass as bass
import concourse.tile as tile
from concourse import bacc, mybir
from concourse.bass import ds, ts
from concourse.bass_utils import run_bass_kernel_spmd

F32 = mybir.dt.float32
BF16 = mybir.dt.bfloat16
FP8 = mybir.dt.float8e4
ACTF = mybir.ActivationFunctionType
DR = mybir.MatmulPerfMode.DoubleRow

# Full-problem dims
BATCH = 2
SEQ = 2048
D_MODEL = 2048
NUM_HEADS = 16
HEAD_DIM = 128
N_CORES = 8
N_GROUPS = 4  # head-groups (tensor parallel)
DG = D_MODEL // N_GROUPS  # 512 = 4 heads per group
SCALE = 1.0 / float(np.sqrt(HEAD_DIM))

QB = 512  # q-block width (= proj seq-slice width)
N_QB = SEQ // QB  # 4
NKD = D_MODEL // 128  # 16 contraction tiles
NDG = DG // 128  # 4 head tiles per group
NSK = SEQ // 128  # 16 seq subtiles

NP_BF16 = ml_dtypes.bfloat16
NP_FP8 = ml_dtypes.float8_e4m3


def _mha_body(ctx, tc, aps):
    nc = tc.nc
    S, D = SEQ, D_MODEL

    xtb = aps["xtb"]
    wqt, wkt, wvt, wot = aps["wqt"], aps["wkt"], aps["wvt"], aps["wot"]
    out = aps["out"]

    consts = ctx.enter_context(tc.tile_pool(name="consts", bufs=1))
    # dummy Exp first: forces the ACT function-table load to queue ahead of
    # the bulk input loads (else early PSUM drains stall on the table load)
    warm = consts.tile([128, 1], F32, name="act_warm")
    nc.vector.memset(warm[:], 0.0)
    nc.scalar.activation(warm[:], warm[:], ACTF.Exp, scale=1.0)
    zb = consts.tile([128, 1], F32, name="zb")
    nc.vector.memset(zb[:], 0.0)
    ones8 = consts.tile([128, 2, 32], FP8, name="ones8")
    nc.vector.memset(ones8[:], 1.0)
    ones_bf = consts.tile([128, 1], BF16, name="ones_bf")
    nc.vector.memset(ones_bf[:], 1.0)
    bq_sb = consts.tile([128, NDG], F32, name="bq_sb")
    bk_sb = consts.tile([128, NDG], F32, name="bk_sb")
    masks_sb = consts.tile([128, 4 * QB], F32, name="masks_sb")
    bo_sb = consts.tile([128, D], F32, name="bo_sb")

    wpool = ctx.enter_context(tc.tile_pool(name="wpool", bufs=1))
    wq_sb = wpool.tile([128, NKD * DG], BF16, name="wq_sb")
    wk_sb = wpool.tile([128, NKD * DG], BF16, name="wk_sb")
    wv_sb = wpool.tile([128, NKD * DG], BF16, name="wv_sb")
    wo_sb = wpool.tile([128, NDG * D], BF16, name="wo_sb")

    res = ctx.enter_context(tc.tile_pool(name="res", bufs=1))
    kt_res = [res.tile([128, S], BF16, name=f"kt{h}") for h in range(NDG)]
    qt_res = [res.tile([128, S], BF16, name=f"qt{h}") for h in range(NDG)]
    ctx_sb = [res.tile([128, S], BF16, name=f"ctx{h}") for h in range(NDG)]
    v_res = res.tile([128, NSK * DG], FP8, name="v_res")
    # bf16 copy of v rows 0-511: early q-rows average few positions, so the
    # fp8 quantization noise does not cancel there — q-block 0 runs bf16
    v16 = res.tile([128, 4 * DG], BF16, name="v16")

    xb_pool = ctx.enter_context(tc.tile_pool(name="xb", bufs=2))
    ex_pool = ctx.enter_context(tc.tile_pool(name="ex", bufs=4))
    exb_pool = ctx.enter_context(tc.tile_pool(name="exb", bufs=3))
    rec_pool = ctx.enter_context(tc.tile_pool(name="rec", bufs=2))
    bc_pool = ctx.enter_context(tc.tile_pool(name="bc", bufs=2))
    ot_pool = ctx.enter_context(tc.tile_pool(name="ot", bufs=3))

    ps_fill = ctx.enter_context(tc.tile_pool(name="ps_fill", bufs=2, space="PSUM"))
    attn_ps = ExitStack()
    ps_sc = attn_ps.enter_context(tc.tile_pool(name="ps_sc", bufs=3, space="PSUM"))
    ps_c = attn_ps.enter_context(tc.tile_pool(name="ps_c", bufs=1, space="PSUM"))
    ps_l = attn_ps.enter_context(tc.tile_pool(name="ps_l", bufs=1, space="PSUM"))
    # epilogue out-proj pool: opened after the attention pools close
    cur = {"fill_pool": ps_fill, "store_eng": nc.gpsimd}

    def v3(t2_lo, t2_n, h):
        # [128, t2_n, 128] pair view of v_res for head h
        return v_res[:].rearrange("p (t f) -> p t f", f=DG)[
            :, ds(t2_lo, t2_n), ts(h, 128)
        ]

    # ---------------- DMA loaders (SP/sync queue) ----------------
    def load_w(dst, src, f, nq, n_k=NKD):
        # dst [128, (k f)] <- src [n_k*128, f], in nq k-chunks
        kq = n_k // nq
        for i in range(nq):
            nc.sync.dma_start(
                dst[:, ds(i * kq * f, kq * f)].rearrange("p (k f) -> p k f", k=kq),
                src[ds(i * kq * 128, kq * 128), :].rearrange(
                    "(k p) f -> p k f", p=128
                ),
            )

    def load_x(pool, src, j, dt, nq, tag):
        # [128, (k QB)] slice j of x^T, in nq k-chunks
        t = pool.tile([128, NKD * QB], dt, tag=tag, name=f"{tag}_sb")
        kq = NKD // nq
        for i in range(nq):
            nc.sync.dma_start(
                t[:, ds(i * kq * QB, kq * QB)].rearrange("p (k f) -> p k f", k=kq),
                src[ds(i * kq * 128, kq * 128), ts(j, QB)].rearrange(
                    "(k p) f -> p k f", p=128
                ),
            )
        return t

    # ---------------- work units ----------------
    def emit_kq_unit(proj, j, m, xb_t, first=None):
        w_sb, res_t, b_sb = (
            (wk_sb, kt_res, bk_sb) if proj == "k" else (wq_sb, qt_res, bq_sb)
        )
        ps = ps_fill.tile([128, QB], F32, tag="fill", name="ps_fill")
        for k in range(NKD):
            if k == 0 and first is not None:
                lhsT, rhs = first[0][:], first[1][:]
            else:
                lhsT = w_sb[:, ds(k * DG + m * 128, 128)]
                rhs = xb_t[:, ts(k, QB)]
            nc.tensor.matmul(
                ps[:],
                lhsT=lhsT,
                rhs=rhs,
                start=(k == 0),
                stop=(k == NKD - 1),
            )
        # bias-add drain on DVE keeps the Activation engine free for exps
        nc.vector.tensor_scalar_add(
            res_t[m][:, ts(j, QB)], ps[:], b_sb[:, ds(m, 1)]
        )

    def emit_v_unit(t, xb_t, s_local):
        ps = ps_fill.tile([128, DG], F32, tag="fill", name="ps_fill")
        for k in range(NKD):
            nc.tensor.matmul(
                ps[:],
                lhsT=xb_t[:, ds(k * QB + s_local * 128, 128)],
                rhs=wv_sb[:, ts(k, DG)],
                start=(k == 0),
                stop=(k == NKD - 1),
            )
        nc.scalar.activation(
            v_res[:, ds(t * DG, DG)], ps[:], ACTF.Identity, bias=zb[:, 0:1]
        )
        if t < 4:
            nc.scalar.activation(
                v16[:, ds(t * DG, DG)], ps[:], ACTF.Identity, bias=zb[:, 0:1]
            )

    def emit_out_unit(m, n):
        ps = cur["fill_pool"].tile([128, QB], F32, tag="fill", name="ps_fill")
        for h in range(NDG):
            nc.tensor.matmul(
                ps[:],
                lhsT=ctx_sb[h][:, ts(m, 128)],
                rhs=wo_sb[:, ds(h * D + n * QB, QB)],
                start=(h == 0),
                stop=(h == NDG - 1),
            )
        ot = ot_pool.tile([128, QB], F32, tag="ot", name="ot")
        nc.vector.tensor_add(ot[:], ps[:], bo_sb[:, ts(n, QB)])
        cur["store_eng"].dma_start(out[ts(m, 128), ts(n, QB)], ot[:])

    # filler queue: (deadline_stage, est_ns, closure). Units with
    # deadline <= current stage are force-emitted at stage start (their
    # results are read by that stage's attention); the rest pace the rounds.
    filler = []

    def push_slice_units(proj, j, xb_t):
        for m in range(NDG):
            filler.append(
                (j, 3413, lambda p=proj, jj=j, mm=m, t=xb_t: emit_kq_unit(p, jj, mm, t))
            )

    def push_v_units(jb, xb_t):
        for s in range(QB // 128):
            filler.append(
                (jb, 3413, lambda t=jb * 4 + s, xt=xb_t, sl=s: emit_v_unit(t, xt, sl))
            )

    def push_out_units(qb):
        for m in range(qb * 4, qb * 4 + 4):
            for n in range(N_QB):
                filler.append((99, 853, lambda mm=m, nn=n: emit_out_unit(mm, nn)))

    def emit_due(stage):
        rest = []
        for d, est, fn in filler:
            if d <= stage:
                fn()
            else:
                rest.append((d, est, fn))
        filler[:] = rest

    # Round pacing: each attention round accrues an allowance of
    # (remaining filler) / (remaining rounds) and pops units while they fit,
    # carrying any remainder so big units never overshoot the round budget.
    sched = {"allowance": 0.0, "rounds_left": 1}

    def emit_filler(_budget=None):
        if not filler:
            return
        est_total = sum(e for _, e, _ in filler)
        sched["allowance"] += est_total / max(1, sched["rounds_left"])
        while filler and filler[0][1] <= sched["allowance"]:
            _, est, fn = filler.pop(0)
            fn()
            sched["allowance"] -= est
        sched["allowance"] = min(sched["allowance"], 4000.0)

    MASK_RANGES = {0: (0, 128), 1: (0, 256), 2: (256, 384), 3: (256, 512)}

    # ---------------- attention for one (qb, head) ----------------
    def emit_attn_head(qb, h, budget_per_round):
        n_pairs = 2 * (qb + 1)
        n_kt = 4 * (qb + 1)
        pc = ps_c.tile([128, QB], F32, tag=f"c{h % 2}", name="ps_c")
        pl = ps_l.tile([32, QB], F32, tag="l", name="ps_l")
        ex_tiles = {}

        def pair_sc(t2):
            # last pair of the q-block covers diagonal offs (2,3): cols 256+
            return 256 if t2 == n_pairs - 1 else 0

        def emit_scores(t2):
            sc = pair_sc(t2)
            w = QB - sc
            if qb == 0:
                ex = exb_pool.tile([128, 2 * QB], BF16, tag="exb", name="exb")
            else:
                ex = ex_pool.tile([128, 2 * QB], FP8, tag="ex", name="ex")
            for i in range(2):
                kt = 2 * t2 + i
                off = kt - (n_kt - 4)
                ps = ps_sc.tile([128, QB], F32, tag="sc", name="ps_sc")
                nc.tensor.matmul(
                    ps[:, ds(sc, w)],
                    lhsT=kt_res[h][:, ts(kt, 128)],
                    rhs=qt_res[h][:, ds(qb * QB + sc, w)],
                    start=True,
                    stop=True,
                )
                if off >= 0:
                    lo, hi = MASK_RANGES[off]
                    nc.vector.tensor_add(
                        ps[:, ds(lo, hi - lo)],
                        ps[:, ds(lo, hi - lo)],
                        masks_sb[:, ds(off * QB + lo, hi - lo)],
                    )
                nc.scalar.activation(
                    ex[:, ds(i * QB + sc, w)],
                    ps[:, ds(sc, w)],
                    ACTF.Exp,
                    scale=SCALE,
                )
            ex_tiles[t2] = ex

        def emit_pv_l(t2):
            sc = pair_sc(t2)
            w = QB - sc
            ex = ex_tiles.pop(t2)
            if qb == 0:
                # bf16 path: per-128 k-subtile matmuls against the bf16 v copy
                for i in range(2):
                    kt = 2 * t2 + i
                    nc.tensor.matmul(
                        pc[:, ds(sc, w)],
                        lhsT=v16[:, ds(kt * DG + h * 128, 128)],
                        rhs=ex[:, ds(i * QB + sc, w)],
                        start=(kt == 0),
                        stop=(kt == 2 * n_pairs - 1),
                        skip_group_check=True,
                    )
                    nc.tensor.matmul(
                        pl[ds(0, 1), ds(sc, w)],
                        lhsT=ones_bf[:],
                        rhs=ex[:, ds(i * QB + sc, w)],
                        start=(kt == 0),
                        stop=(kt == 2 * n_pairs - 1),
                        skip_group_check=True,
                    )
                return
            exv = ex[:].rearrange("p (two f) -> p two f", two=2)[:, :, ds(sc, w)]
            nc.tensor.matmul(
                pc[:, ds(sc, w)],
                lhsT=v3(2 * t2, 2, h),
                rhs=exv,
                start=(t2 == 0),
                stop=(t2 == n_pairs - 1),
                perf_mode=DR,
                skip_group_check=True,
            )
            nc.tensor.matmul(
                pl[:, ds(sc, w)],
                lhsT=ones8[:],
                rhs=exv,
                start=(t2 == 0),
                stop=(t2 == n_pairs - 1),
                perf_mode=DR,
                skip_group_check=True,
            )

        for t2 in range(n_pairs):
            emit_scores(t2)
            emit_filler()
            emit_pv_l(t2)
            sched["rounds_left"] -= 1
        rec = rec_pool.tile([1, QB], F32, tag="r", name="rec")
        nc.vector.reciprocal(rec[:], pl[ds(0, 1), :])
        bc = bc_pool.tile([128, QB], F32, tag="bc", name="bc")
        nc.gpsimd.partition_broadcast(bc[:], rec[:])
        nc.vector.tensor_mul(ctx_sb[h][:, ts(qb, QB)], pc[:], bc[:])

    # ---------------- prologue DMAs ----------------
    # tiny first chunks: the very first matmul (k-proj slice0 m0 k0) can
    # start as soon as these two small DMAs land (~2.5us) instead of
    # waiting for the full quarter loads
    wk_first = consts.tile([128, 128], BF16, name="wk_first")
    xb_first = consts.tile([128, QB], BF16, name="xb_first")
    nc.sync.dma_start(wk_first[:], wkt[0:128, 0:128])
    nc.sync.dma_start(xb_first[:], xtb[0:128, 0:QB])
    nc.sync.dma_start(bq_sb[:], aps["bq"])
    nc.sync.dma_start(bk_sb[:], aps["bk"])
    # interleave wk / x slice-0 quarters so the first k-proj unit starts early
    xb_t = {}
    xb_t[0] = xb_pool.tile([128, NKD * QB], BF16, tag="xb", name="xb_sb")
    for i in range(4):
        nc.sync.dma_start(
            wk_sb[:, ds(i * 4 * DG, 4 * DG)].rearrange("p (k f) -> p k f", k=4),
            wkt[ds(i * 4 * 128, 4 * 128), :].rearrange("(k p) f -> p k f", p=128),
        )
        nc.sync.dma_start(
            xb_t[0][:, ds(i * 4 * QB, 4 * QB)].rearrange("p (k f) -> p k f", k=4),
            xtb[ds(i * 4 * 128, 4 * 128), 0:QB].rearrange("(k p) f -> p k f", p=128),
        )
    load_w(wq_sb, wqt, DG, 2)
    load_w(wv_sb, wvt, DG, 2)
    xb_t[1] = load_x(xb_pool, xtb, 1, BF16, 2, "xb")
    nc.sync.dma_start(
        masks_sb[:].rearrange("p (i f) -> p i f", i=4),
        aps["masks"].rearrange("i p f -> p i f"),
    )
    nc.sync.dma_start(bo_sb[:], aps["bo"])
    load_w(wo_sb, wot, D, 1, n_k=NDG)

    # ---------------- prologue compute: slice 0 proj + v block 0 ----------
    emit_kq_unit("k", 0, 0, xb_t[0], first=(wk_first, xb_first))
    for m in range(1, NDG):
        emit_kq_unit("k", 0, m, xb_t[0])
    for m in range(NDG):
        emit_kq_unit("q", 0, m, xb_t[0])
    for s in range(4):
        emit_v_unit(s, xb_t[0], s)

    # ---------------- main interleaved schedule ----------------
    total_rounds = sum(2 * (qb + 1) * NDG for qb in range(N_QB))  # 80
    sched["rounds_left"] = total_rounds

    for qb in range(N_QB):
        if qb + 1 < N_QB:
            if qb + 1 not in xb_t:
                xb_t[qb + 1] = load_x(xb_pool, xtb, qb + 1, BF16, 2, "xb")
            push_slice_units("k", qb + 1, xb_t[qb + 1])
            push_slice_units("q", qb + 1, xb_t[qb + 1])
            push_v_units(qb + 1, xb_t[qb + 1])
        emit_due(qb)  # everything this stage's attention reads must be emitted
        if qb == N_QB - 1:
            # last stage: no loads remain, so stores can use the SP queue
            cur["store_eng"] = nc.sync
        for h in range(NDG):
            emit_attn_head(qb, h, None)
        push_out_units(qb)

    # epilogue: attention PSUM banks are free now — use a wide pool so the
    # remaining out-proj tiles pipeline without drain stalls
    attn_ps.close()
    ps_epi = ctx.enter_context(tc.tile_pool(name="ps_epi", bufs=5, space="PSUM"))
    cur["fill_pool"] = ps_epi
    while filler:
        _, _, fn = filler.pop(0)
        fn()


def build_program(enable_asserts=False):
    nc = bacc.Bacc(
        "TRN2",
        target_bir_lowering=False,
        debug=False,
        enable_asserts=enable_asserts,
        num_devices=N_CORES,
    )
    aps = {
        "xtb": nc.dram_tensor("xtb", [D_MODEL, SEQ], BF16, kind="ExternalInput").ap(),
        "wqt": nc.dram_tensor("wqt", [D_MODEL, DG], BF16, kind="ExternalInput").ap(),
        "wkt": nc.dram_tensor("wkt", [D_MODEL, DG], BF16, kind="ExternalInput").ap(),
        "wvt": nc.dram_tensor("wvt", [D_MODEL, DG], BF16, kind="ExternalInput").ap(),
        "wot": nc.dram_tensor("wot", [DG, D_MODEL], BF16, kind="ExternalInput").ap(),
        "bq": nc.dram_tensor("bq", [128, NDG], F32, kind="ExternalInput").ap(),
        "bk": nc.dram_tensor("bk", [128, NDG], F32, kind="ExternalInput").ap(),
        "bo": nc.dram_tensor("bo", [128, D_MODEL], F32, kind="ExternalInput").ap(),
        "masks": nc.dram_tensor("masks", [4, 128, QB], F32, kind="ExternalInput").ap(),
        "out": nc.dram_tensor("out", [SEQ, D_MODEL], F32, kind="ExternalOutput").ap(),
    }
    with tile.TileContext(nc) as tc:
        with ExitStack() as ctx:
            _mha_body(ctx, tc, aps)
    nc.compile()
    return nc


def make_masks():
    """Additive causal masks: 0 where k<=q, -1e30 where masked."""
    i = np.arange(4)[:, None, None]
    p = np.arange(128)[None, :, None]
    f = np.arange(QB)[None, None, :]
    keep = (i * 128 + p) <= f
    return np.where(keep, 0.0, -1e30).astype(np.float32)


def shard_inputs(x, wq, bq, wk, bk, wv, bv, wo, bo):
    """Build the 8 per-core input maps (host-side layout + dtype prep)."""
    masks = make_masks()
    x = np.asarray(x, np.float32)
    wq, wk, wv, wo = (np.asarray(a, np.float32) for a in (wq, wk, wv, wo))
    bq, bk, bv, bo = (np.asarray(a, np.float32) for a in (bq, bk, bv, bo))
    xts = [np.ascontiguousarray(x[b].T) for b in range(BATCH)]
    xtbs = [t.astype(NP_BF16) for t in xts]
    in_maps = []
    for c in range(N_CORES):
        b, g = divmod(c, N_GROUPS)
        sl = slice(g * DG, (g + 1) * DG)
        # fold the v bias through the output projection: out += wo[:,sl] @ bv[sl]
        bo_eff = wo[:, sl].astype(np.float64) @ bv[sl].astype(np.float64)
        if g == 0:
            bo_eff = bo_eff + bo
        bo_bc = np.ascontiguousarray(
            np.broadcast_to(bo_eff.astype(np.float32), (128, D_MODEL))
        )
        in_maps.append(
            {
                "xtb": xtbs[b],
                "wqt": np.ascontiguousarray(wq[sl].T).astype(NP_BF16),
                "wkt": np.ascontiguousarray(wk[sl].T).astype(NP_BF16),
                "wvt": np.ascontiguousarray(wv[sl].T).astype(NP_BF16),
                "wot": np.ascontiguousarray(wo[:, sl].T).astype(NP_BF16),
                "bq": np.ascontiguousarray(bq[sl].reshape(-1, 128).T),
                "bk": np.ascontiguousarray(bk[sl].reshape(-1, 128).T),
                "bo": bo_bc,
                "masks": masks,
            }
        )
    return in_maps


_NC_CACHE = {}


def get_program():
    if "nc" not in _NC_CACHE:
        _NC_CACHE["nc"] = build_program()
    return _NC_CACHE["nc"]


def run_sharded(inputs, trace=False):
    nc = get_program()
    in_maps = shard_inputs(**inputs)
    res = run_bass_kernel_spmd(nc, in_maps, list(range(N_CORES)), trace=trace)
    full = np.empty((BATCH, SEQ, D_MODEL), np.float32)
    for b in range(BATCH):
        acc = res.results[b * N_GROUPS]["out"].copy()
        for g in range(1, N_GROUPS):
            acc += res.results[b * N_GROUPS + g]["out"]
        full[b] = acc
    return full, res


def kernel(**inputs):
    out, _ = run_sharded(inputs, trace=False)
    return out


# revision 52
# speedup vs baseline: 1.2701x; 1.1216x over previous
"""Multi-head causal attention block on 8 Trainium2 NeuronCores.

Sharding: tensor-parallel over heads (4 groups of 4 heads) x data-parallel
over batch (2). Core c -> (batch b=c//4, head-group g=c%4). Each core
computes q/k/v projections for its head group, causal attention for its 4
heads, and a partial output projection; the host sums the 4 partials per
batch.

Single fused pipeline: attention (scores bf16, PV / softmax-sum in fp8
DoubleRow), q/k projections (bf16), v projection (fp8 DoubleRow) and the
output projection (bf16) are interleaved per q-block so the exp
(Activation engine) work hides under projection matmuls and the PE never
idles. v stays resident in SBUF (no DRAM round trip); the v bias is folded
into the output bias host-side (wo @ bv is a constant).

Self-contained: hardcodes shapes for the 2x2048x2048, 16-head problem.
"""

from contextlib import ExitStack

import numpy as np
import ml_dtypes

import concourse.bass as bass
import concourse.tile as tile
from concourse import bacc, mybir
from concourse.bass import ds, ts
from concourse.bass_utils import run_bass_kernel_spmd

F32 = mybir.dt.float32
BF16 = mybir.dt.bfloat16
FP8 = mybir.dt.float8e4
ACTF = mybir.ActivationFunctionType
DR = mybir.MatmulPerfMode.DoubleRow

# Full-problem dims
BATCH = 2
SEQ = 2048
D_MODEL = 2048
NUM_HEADS = 16
HEAD_DIM = 128
N_CORES = 8
N_GROUPS = 4  # head-groups (tensor parallel)
DG = D_MODEL // N_GROUPS  # 512 = 4 heads per group
SCALE = 1.0 / float(np.sqrt(HEAD_DIM))

QB = 512  # q-block width (= proj seq-slice width)
N_QB = SEQ // QB  # 4
NKD = D_MODEL // 128  # 16 contraction tiles
NDG = DG // 128  # 4 head tiles per group
NSK = SEQ // 128  # 16 seq subtiles

NP_BF16 = ml_dtypes.bfloat16
NP_FP8 = ml_dtypes.float8_e4m3


def _mha_body(ctx, tc, aps):
    nc = tc.nc
    S, D = SEQ, D_MODEL

    xtb, xt8 = aps["xtb"], aps["xt8"]
    wqt, wkt, wvt, wvt8, wot = (
        aps["wqt"], aps["wkt"], aps["wvt"], aps["wvt8"], aps["wot"]
    )
    out = aps["out"]

    consts = ctx.enter_context(tc.tile_pool(name="consts", bufs=1))
    # dummy Exp first: forces the ACT function-table load to queue ahead of
    # the bulk input loads (else early PSUM drains stall on the table load)
    warm = consts.tile([128, 1], F32, name="act_warm")
    nc.vector.memset(warm[:], 0.0)
    nc.scalar.activation(warm[:], warm[:], ACTF.Exp, scale=1.0)
    zb = consts.tile([128, 1], F32, name="zb")
    nc.vector.memset(zb[:], 0.0)
    ones8 = consts.tile([128, 2, 32], FP8, name="ones8")
    nc.vector.memset(ones8[:], 1.0)
    ones_bf = consts.tile([128, 1], BF16, name="ones_bf")
    nc.vector.memset(ones_bf[:], 1.0)
    bq_sb = consts.tile([128, NDG], F32, name="bq_sb")
    bk_sb = consts.tile([128, NDG], F32, name="bk_sb")
    masks_sb = consts.tile([128, 4 * QB], F32, name="masks_sb")
    bo_sb = consts.tile([128, D], F32, name="bo_sb")

    wpool = ctx.enter_context(tc.tile_pool(name="wpool", bufs=1))
    wq_sb = wpool.tile([128, NKD * DG], BF16, name="wq_sb")
    wk_sb = wpool.tile([128, NKD * DG], BF16, name="wk_sb")
    wv_sb = wpool.tile([128, NKD * DG], BF16, name="wv_sb")
    wv8_sb = wpool.tile([128, NKD * DG], FP8, name="wv8_sb")
    wo_sb = wpool.tile([128, NDG * D], BF16, name="wo_sb")

    res = ctx.enter_context(tc.tile_pool(name="res", bufs=1))
    kt_res = [res.tile([128, S], BF16, name=f"kt{h}") for h in range(NDG)]
    qt_res = [res.tile([128, S], BF16, name=f"qt{h}") for h in range(NDG)]
    ctx_sb = [res.tile([128, S], BF16, name=f"ctx{h}") for h in range(NDG)]
    v_res = res.tile([128, NSK * DG], FP8, name="v_res")
    # bf16 copy of v rows 0-511: early q-rows average few positions, so the
    # fp8 quantization noise does not cancel there — q-block 0 runs bf16
    v16 = res.tile([128, 4 * DG], BF16, name="v16")

    xb_pool = ctx.enter_context(tc.tile_pool(name="xb", bufs=2))
    x8_pool = ctx.enter_context(tc.tile_pool(name="x8", bufs=1))
    ex_pool = ctx.enter_context(tc.tile_pool(name="ex", bufs=3))
    exb_pool = ctx.enter_context(tc.tile_pool(name="exb", bufs=2))
    rec_pool = ctx.enter_context(tc.tile_pool(name="rec", bufs=2))
    bc_pool = ctx.enter_context(tc.tile_pool(name="bc", bufs=2))
    ot_pool = ctx.enter_context(tc.tile_pool(name="ot", bufs=3))

    ps_fill = ctx.enter_context(tc.tile_pool(name="ps_fill", bufs=2, space="PSUM"))
    attn_ps = ExitStack()
    ps_sc = attn_ps.enter_context(tc.tile_pool(name="ps_sc", bufs=3, space="PSUM"))
    ps_c = attn_ps.enter_context(tc.tile_pool(name="ps_c", bufs=1, space="PSUM"))
    ps_l = attn_ps.enter_context(tc.tile_pool(name="ps_l", bufs=1, space="PSUM"))
    # epilogue out-proj pool: opened after the attention pools close
    cur = {"fill_pool": ps_fill, "store_eng": nc.gpsimd}

    def v3(t2_lo, t2_n, h):
        # [128, t2_n, 128] pair view of v_res for head h
        return v_res[:].rearrange("p (t f) -> p t f", f=DG)[
            :, ds(t2_lo, t2_n), ts(h, 128)
        ]

    # ---------------- DMA loaders (SP/sync queue) ----------------
    def load_w(dst, src, f, nq, n_k=NKD):
        # dst [128, (k f)] <- src [n_k*128, f], in nq k-chunks
        kq = n_k // nq
        for i in range(nq):
            nc.sync.dma_start(
                dst[:, ds(i * kq * f, kq * f)].rearrange("p (k f) -> p k f", k=kq),
                src[ds(i * kq * 128, kq * 128), :].rearrange(
                    "(k p) f -> p k f", p=128
                ),
            )

    def load_x(pool, src, j, dt, nq, tag):
        # [128, (k QB)] slice j of x^T, in nq k-chunks
        t = pool.tile([128, NKD * QB], dt, tag=tag, name=f"{tag}_sb")
        kq = NKD // nq
        for i in range(nq):
            nc.sync.dma_start(
                t[:, ds(i * kq * QB, kq * QB)].rearrange("p (k f) -> p k f", k=kq),
                src[ds(i * kq * 128, kq * 128), ts(j, QB)].rearrange(
                    "(k p) f -> p k f", p=128
                ),
            )
        return t

    # ---------------- work units ----------------
    def emit_kq_unit(proj, j, m, xb_t, first=None):
        w_sb, res_t, b_sb = (
            (wk_sb, kt_res, bk_sb) if proj == "k" else (wq_sb, qt_res, bq_sb)
        )
        ps = ps_fill.tile([128, QB], F32, tag="fill", name="ps_fill")
        for k in range(NKD):
            if k == 0 and first is not None:
                lhsT, rhs = first[0][:], first[1][:]
            else:
                lhsT = w_sb[:, ds(k * DG + m * 128, 128)]
                rhs = xb_t[:, ts(k, QB)]
            nc.tensor.matmul(
                ps[:],
                lhsT=lhsT,
                rhs=rhs,
                start=(k == 0),
                stop=(k == NKD - 1),
            )
        # bias-add drain on DVE keeps the Activation engine free for exps
        nc.vector.tensor_scalar_add(
            res_t[m][:, ts(j, QB)], ps[:], b_sb[:, ds(m, 1)]
        )

    def emit_v_unit(t, src_t, s_local):
        ps = ps_fill.tile([128, DG], F32, tag="fill", name="ps_fill")
        if t < 4:
            # rows 0-511 need precise v (early q-rows can't average away
            # fp8 noise): bf16 projection, dual-drained to bf16 + fp8
            for k in range(NKD):
                nc.tensor.matmul(
                    ps[:],
                    lhsT=src_t[:, ds(k * QB + s_local * 128, 128)],
                    rhs=wv_sb[:, ts(k, DG)],
                    start=(k == 0),
                    stop=(k == NKD - 1),
                )
            nc.scalar.activation(
                v16[:, ds(t * DG, DG)], ps[:], ACTF.Identity, bias=zb[:, 0:1]
            )
        else:
            x8v = src_t[:].rearrange("p (k s) -> p k s", k=NKD)
            wvv = wv8_sb[:].rearrange("p (k f) -> p k f", k=NKD)
            for kp in range(NKD // 2):
                nc.tensor.matmul(
                    ps[:],
                    lhsT=x8v[:, ds(2 * kp, 2), ds(s_local * 128, 128)],
                    rhs=wvv[:, ds(2 * kp, 2), :],
                    start=(kp == 0),
                    stop=(kp == NKD // 2 - 1),
                    perf_mode=DR,
                )
        nc.scalar.activation(
            v_res[:, ds(t * DG, DG)], ps[:], ACTF.Identity, bias=zb[:, 0:1]
        )

    def emit_out_unit(m, n):
        ps = cur["fill_pool"].tile([128, QB], F32, tag="fill", name="ps_fill")
        for h in range(NDG):
            nc.tensor.matmul(
                ps[:],
                lhsT=ctx_sb[h][:, ts(m, 128)],
                rhs=wo_sb[:, ds(h * D + n * QB, QB)],
                start=(h == 0),
                stop=(h == NDG - 1),
            )
        ot = ot_pool.tile([128, QB], BF16, tag="ot", name="ot")
        nc.vector.tensor_add(ot[:], ps[:], bo_sb[:, ts(n, QB)])
        cur["store_eng"].dma_start(out[ts(m, 128), ts(n, QB)], ot[:])

    # filler queue: (deadline_stage, est_ns, closure). Units with
    # deadline <= current stage are force-emitted at stage start (their
    # results are read by that stage's attention); the rest pace the rounds.
    filler = []

    def push_slice_units(proj, j, xb_t):
        for m in range(NDG):
            filler.append(
                (j, 3413, lambda p=proj, jj=j, mm=m, t=xb_t: emit_kq_unit(p, jj, mm, t))
            )

    def push_v_units(jb, src_t):
        est = 3413 if jb == 0 else 853
        for s in range(QB // 128):
            filler.append(
                (jb, est, lambda t=jb * 4 + s, xt=src_t, sl=s: emit_v_unit(t, xt, sl))
            )

    def push_out_units(qb):
        for m in range(qb * 4, qb * 4 + 4):
            for n in range(N_QB):
                filler.append((99, 853, lambda mm=m, nn=n: emit_out_unit(mm, nn)))

    def emit_due(stage):
        rest = []
        for d, est, fn in filler:
            if d <= stage:
                fn()
            else:
                rest.append((d, est, fn))
        filler[:] = rest

    # Round pacing: each attention round accrues an allowance of
    # (remaining filler) / (remaining rounds) and pops units while they fit,
    # carrying any remainder so big units never overshoot the round budget.
    sched = {"allowance": 0.0, "rounds_left": 1}

    def emit_filler(_budget=None):
        if not filler:
            return
        est_total = sum(e for _, e, _ in filler)
        sched["allowance"] += est_total / max(1, sched["rounds_left"])
        while filler and filler[0][1] <= sched["allowance"]:
            _, est, fn = filler.pop(0)
            fn()
            sched["allowance"] -= est
        sched["allowance"] = min(sched["allowance"], 4000.0)

    MASK_RANGES = {0: (0, 128), 1: (0, 256), 2: (256, 384), 3: (256, 512)}

    # ---------------- attention for one (qb, head) ----------------
    def emit_attn_head(qb, h, budget_per_round):
        n_pairs = 2 * (qb + 1)
        n_kt = 4 * (qb + 1)
        pc = ps_c.tile([128, QB], F32, tag=f"c{h % 2}", name="ps_c")
        pl = ps_l.tile([32, QB], F32, tag="l", name="ps_l")
        ex_tiles = {}

        def pair_sc(t2):
            # last pair of the q-block covers diagonal offs (2,3): cols 256+
            return 256 if t2 == n_pairs - 1 else 0

        def emit_scores(t2):
            sc = pair_sc(t2)
            w = QB - sc
            if qb == 0:
                ex = exb_pool.tile([128, 2 * QB], BF16, tag="exb", name="exb")
            else:
                ex = ex_pool.tile([128, 2 * QB], FP8, tag="ex", name="ex")
            for i in range(2):
                kt = 2 * t2 + i
                off = kt - (n_kt - 4)
                ps = ps_sc.tile([128, QB], F32, tag="sc", name="ps_sc")
                nc.tensor.matmul(
                    ps[:, ds(sc, w)],
                    lhsT=kt_res[h][:, ts(kt, 128)],
                    rhs=qt_res[h][:, ds(qb * QB + sc, w)],
                    start=True,
                    stop=True,
                )
                if off >= 0:
                    lo, hi = MASK_RANGES[off]
                    nc.vector.tensor_add(
                        ps[:, ds(lo, hi - lo)],
                        ps[:, ds(lo, hi - lo)],
                        masks_sb[:, ds(off * QB + lo, hi - lo)],
                    )
                nc.scalar.activation(
                    ex[:, ds(i * QB + sc, w)],
                    ps[:, ds(sc, w)],
                    ACTF.Exp,
                    scale=SCALE,
                )
            ex_tiles[t2] = ex

        def emit_pv_l(t2):
            sc = pair_sc(t2)
            w = QB - sc
            ex = ex_tiles.pop(t2)
            if qb == 0:
                # bf16 path: per-128 k-subtile matmuls against the bf16 v copy
                for i in range(2):
                    kt = 2 * t2 + i
                    nc.tensor.matmul(
                        pc[:, ds(sc, w)],
                        lhsT=v16[:, ds(kt * DG + h * 128, 128)],
                        rhs=ex[:, ds(i * QB + sc, w)],
                        start=(kt == 0),
                        stop=(kt == 2 * n_pairs - 1),
                        skip_group_check=True,
                    )
                    nc.tensor.matmul(
                        pl[ds(0, 1), ds(sc, w)],
                        lhsT=ones_bf[:],
                        rhs=ex[:, ds(i * QB + sc, w)],
                        start=(kt == 0),
                        stop=(kt == 2 * n_pairs - 1),
                        skip_group_check=True,
                    )
                return
            exv = ex[:].rearrange("p (two f) -> p two f", two=2)[:, :, ds(sc, w)]
            nc.tensor.matmul(
                pc[:, ds(sc, w)],
                lhsT=v3(2 * t2, 2, h),
                rhs=exv,
                start=(t2 == 0),
                stop=(t2 == n_pairs - 1),
                perf_mode=DR,
                skip_group_check=True,
            )
            nc.tensor.matmul(
                pl[:, ds(sc, w)],
                lhsT=ones8[:],
                rhs=exv,
                start=(t2 == 0),
                stop=(t2 == n_pairs - 1),
                perf_mode=DR,
                skip_group_check=True,
            )

        for t2 in range(n_pairs):
            emit_scores(t2)
            emit_filler()
            emit_pv_l(t2)
            sched["rounds_left"] -= 1
        rec = rec_pool.tile([1, QB], F32, tag="r", name="rec")
        nc.vector.reciprocal(rec[:], pl[ds(0, 1), :])
        bc = bc_pool.tile([128, QB], F32, tag="bc", name="bc")
        nc.gpsimd.partition_broadcast(bc[:], rec[:])
        nc.vector.tensor_mul(ctx_sb[h][:, ts(qb, QB)], pc[:], bc[:])

    # ---------------- prologue DMAs ----------------
    # tiny first chunks: the very first matmul (k-proj slice0 m0 k0) can
    # start as soon as these two small DMAs land (~2.5us) instead of
    # waiting for the full quarter loads
    wk_first = consts.tile([128, 128], BF16, name="wk_first")
    xb_first = consts.tile([128, QB], BF16, name="xb_first")
    nc.sync.dma_start(wk_first[:], wkt[0:128, 0:128])
    nc.sync.dma_start(xb_first[:], xtb[0:128, 0:QB])
    nc.sync.dma_start(bq_sb[:], aps["bq"])
    nc.sync.dma_start(bk_sb[:], aps["bk"])
    # interleave wk / x slice-0 quarters so the first k-proj unit starts early
    xb_t = {}
    xb_t[0] = xb_pool.tile([128, NKD * QB], BF16, tag="xb", name="xb_sb")
    for i in range(4):
        nc.sync.dma_start(
            wk_sb[:, ds(i * 4 * DG, 4 * DG)].rearrange("p (k f) -> p k f", k=4),
            wkt[ds(i * 4 * 128, 4 * 128), :].rearrange("(k p) f -> p k f", p=128),
        )
        nc.sync.dma_start(
            xb_t[0][:, ds(i * 4 * QB, 4 * QB)].rearrange("p (k f) -> p k f", k=4),
            xtb[ds(i * 4 * 128, 4 * 128), 0:QB].rearrange("(k p) f -> p k f", p=128),
        )
    load_w(wq_sb, wqt, DG, 2)
    load_w(wv_sb, wvt, DG, 2)
    load_w(wv8_sb, wvt8, DG, 2)
    xb_t[1] = load_x(xb_pool, xtb, 1, BF16, 2, "xb")
    nc.sync.dma_start(
        masks_sb[:].rearrange("p (i f) -> p i f", i=4),
        aps["masks"].rearrange("i p f -> p i f"),
    )
    nc.sync.dma_start(bo_sb[:], aps["bo"])
    load_w(wo_sb, wot, D, 1, n_k=NDG)

    # ---------------- prologue compute: slice 0 proj + v block 0 ----------
    emit_kq_unit("k", 0, 0, xb_t[0], first=(wk_first, xb_first))
    for m in range(1, NDG):
        emit_kq_unit("k", 0, m, xb_t[0])
    for m in range(NDG):
        emit_kq_unit("q", 0, m, xb_t[0])
    for s in range(4):
        emit_v_unit(s, xb_t[0], s)

    # ---------------- main interleaved schedule ----------------
    total_rounds = sum(2 * (qb + 1) * NDG for qb in range(N_QB))  # 80
    sched["rounds_left"] = total_rounds

    for qb in range(N_QB):
        # force-emit everything this stage's attention reads FIRST: with
        # x8_pool bufs=1 the next x8 load may only be emitted after all
        # readers of the previous x8 tile (this stage's v units) exist
        emit_due(qb)
        if qb + 1 < N_QB:
            if qb + 1 not in xb_t:
                xb_t[qb + 1] = load_x(xb_pool, xtb, qb + 1, BF16, 2, "xb")
            push_slice_units("k", qb + 1, xb_t[qb + 1])
            push_slice_units("q", qb + 1, xb_t[qb + 1])
            x8_t = load_x(x8_pool, xt8, qb + 1, FP8, 2, "x8")
            push_v_units(qb + 1, x8_t)
        if qb == N_QB - 1:
            # last stage: no loads remain, so stores can use the SP queue
            cur["store_eng"] = nc.sync
        for h in range(NDG):
            emit_attn_head(qb, h, None)
        push_out_units(qb)

    # epilogue: attention PSUM banks are free now — use a wide pool so the
    # remaining out-proj tiles pipeline without drain stalls
    attn_ps.close()
    ps_epi = ctx.enter_context(tc.tile_pool(name="ps_epi", bufs=5, space="PSUM"))
    cur["fill_pool"] = ps_epi
    while filler:
        _, _, fn = filler.pop(0)
        fn()


def build_program(enable_asserts=False):
    nc = bacc.Bacc(
        "TRN2",
        target_bir_lowering=False,
        debug=False,
        enable_asserts=enable_asserts,
        num_devices=N_CORES,
    )
    aps = {
        "xtb": nc.dram_tensor("xtb", [D_MODEL, SEQ], BF16, kind="ExternalInput").ap(),
        "xt8": nc.dram_tensor("xt8", [D_MODEL, SEQ], FP8, kind="ExternalInput").ap(),
        "wqt": nc.dram_tensor("wqt", [D_MODEL, DG], BF16, kind="ExternalInput").ap(),
        "wkt": nc.dram_tensor("wkt", [D_MODEL, DG], BF16, kind="ExternalInput").ap(),
        "wvt": nc.dram_tensor("wvt", [D_MODEL, DG], BF16, kind="ExternalInput").ap(),
        "wvt8": nc.dram_tensor("wvt8", [D_MODEL, DG], FP8, kind="ExternalInput").ap(),
        "wot": nc.dram_tensor("wot", [DG, D_MODEL], BF16, kind="ExternalInput").ap(),
        "bq": nc.dram_tensor("bq", [128, NDG], F32, kind="ExternalInput").ap(),
        "bk": nc.dram_tensor("bk", [128, NDG], F32, kind="ExternalInput").ap(),
        "bo": nc.dram_tensor("bo", [128, D_MODEL], F32, kind="ExternalInput").ap(),
        "masks": nc.dram_tensor("masks", [4, 128, QB], F32, kind="ExternalInput").ap(),
        "out": nc.dram_tensor("out", [SEQ, D_MODEL], BF16, kind="ExternalOutput").ap(),
    }
    with tile.TileContext(nc) as tc:
        with ExitStack() as ctx:
            _mha_body(ctx, tc, aps)
    nc.compile()
    return nc


def make_masks():
    """Additive causal masks: 0 where k<=q, -1e30 where masked."""
    i = np.arange(4)[:, None, None]
    p = np.arange(128)[None, :, None]
    f = np.arange(QB)[None, None, :]
    keep = (i * 128 + p) <= f
    return np.where(keep, 0.0, -1e30).astype(np.float32)


def shard_inputs(x, wq, bq, wk, bk, wv, bv, wo, bo):
    """Build the 8 per-core input maps (host-side layout + dtype prep)."""
    masks = make_masks()
    x = np.asarray(x, np.float32)
    wq, wk, wv, wo = (np.asarray(a, np.float32) for a in (wq, wk, wv, wo))
    bq, bk, bv, bo = (np.asarray(a, np.float32) for a in (bq, bk, bv, bo))
    xts = [np.ascontiguousarray(x[b].T) for b in range(BATCH)]
    xtbs = [t.astype(NP_BF16) for t in xts]
    xt8s = [t.astype(NP_FP8) for t in xts]
    in_maps = []
    for c in range(N_CORES):
        b, g = divmod(c, N_GROUPS)
        sl = slice(g * DG, (g + 1) * DG)
        # fold the v bias through the output projection: out += wo[:,sl] @ bv[sl]
        bo_eff = wo[:, sl].astype(np.float64) @ bv[sl].astype(np.float64)
        if g == 0:
            bo_eff = bo_eff + bo
        bo_bc = np.ascontiguousarray(
            np.broadcast_to(bo_eff.astype(np.float32), (128, D_MODEL))
        )
        in_maps.append(
            {
                "xtb": xtbs[b],
                "xt8": xt8s[b],
                "wqt": np.ascontiguousarray(wq[sl].T).astype(NP_BF16),
                "wkt": np.ascontiguousarray(wk[sl].T).astype(NP_BF16),
                "wvt": np.ascontiguousarray(wv[sl].T).astype(NP_BF16),
                "wvt8": np.ascontiguousarray(wv[sl].T).astype(NP_FP8),
                "wot": np.ascontiguousarray(wo[:, sl].T).astype(NP_BF16),
                "bq": np.ascontiguousarray(bq[sl].reshape(-1, 128).T),
                "bk": np.ascontiguousarray(bk[sl].reshape(-1, 128).T),
                "bo": bo_bc,
                "masks": masks,
            }
        )
    return in_maps


_NC_CACHE = {}


def get_program():
    if "nc" not in _NC_CACHE:
        _NC_CACHE["nc"] = build_program()
    return _NC_CACHE["nc"]


def run_sharded(inputs, trace=False):
    nc = get_program()
    in_maps = shard_inputs(**inputs)
    res = run_bass_kernel_spmd(nc, in_maps, list(range(N_CORES)), trace=trace)
    full = np.empty((BATCH, SEQ, D_MODEL), np.float32)
    for b in range(BATCH):
        acc = res.results[b * N_GROUPS]["out"].astype(np.float32)
        for g in range(1, N_GROUPS):
            acc += res.results[b * N_GROUPS + g]["out"].astype(np.float32)
        full[b] = acc
    return full, res


def kernel(**inputs):
    out, _ = run_sharded(inputs, trace=False)
    return out
